# revision 40
# baseline (speedup 1.0000x reference)
"""Trainium2 Bass kernel for nn_Net_75282186764473.

Math: reference pat() returns zm + stop_gradient(ze - zm) == ze numerically;
the forward pass is 5 explicit-Euler steps of the 'experiment' dynamics per
stage, twice:  q' = p ; p' = sin(1.1 q) @ (c2q(C) + Qn - I) + e.
With u = 1.1 q, g_n = sin(u_n) @ W + eb  (W, eb scaled by 1.1*DT^2):
    u2 = u0 + g0 ; u3 = u0 + 3 g0 ; u5 = u0 + 7 g0 + 2 g2 + g3
so each stage needs sins at u0, u2, u3 and weighted passes {1,6,2,1}*W:
the 6W pass tops the g0 PSUM bank up to 7g0 after its last read, then the
2W/1W passes accumulate 2g2 + g3 into the same bank.

All state lives in v = u/pi units so the range wrap is 'bound +-1,
period 2', which two custom DVE ops implement with zero constant slots:
    ADD_WRAP_V  : out = wrap(in0 + in1)
    ADD2_WRAP_V : out = wrap(2*in0 + in1)
(u3 = wrap(u2_wrapped + 2*g0) is exact mod 2.)  sin(u) = Act Sin with
scale=pi on the v-state, so the table sin stays exact; weights carry the
1/pi.  e-biases ride as an extra contraction row driven by a constant
0.5 in the wide state (every wrap maps 0.5 -> 0.5, every sin -> 1.0).

Layout: everything is wide-packed [128, 2, 512] per batch tile -- slot 0
= node rows 0:128, slot 1 = rows 128:196 + driver/class/pad rows -- so
each wrap is ONE wide DVE op over a merged 2-bank PSUM tile and each sin
is ONE wide Act op.  Stage 2 permutes the weights (_permute_v) so its
slot-1 contraction is [state 68 | e-row | class 10] and its targets are
[state 68 | dummy | class 10]: the class-node zeros fall out of zeroed
bank rows + zero input padding automatically.  Per tile: 26 matmuls
(512-col, bf16), 5 wide wraps (DVE), 5 wide sins + 1 copy (Act).
sin(1.1 x) is precomputed on the host and shipped as bf16 next to the
fp32 v0 state.  Stage-1 emission runs two tiles ahead of stage-2
(software pipelining) so the in-order engine queues stay fed.

Sharding: pure batch data-parallel across 8 cores (8192 rows each); the
outputs are PSUM slot-1 partitions 69:79 of the stage-2 bank, scaled by
pi/1.1 on the host.
"""

import ml_dtypes
import numpy as np

import concourse.bacc as bacc
import concourse.bass as bass
import concourse.mybir as mybir
import concourse.tile as tile
from concourse.bass_utils import run_bass_kernel_spmd
from concourse.dve_ops import (
    CUSTOM_DVE_SPECS,
    OPS,
    DveOp,
    _SUB_OPCODE_FOR_NAME,
)
from concourse.dve_spec import One, Spec, Src0, Src1, Zero, lower
from concourse.dve_uop import DveOpSpec

AF = mybir.ActivationFunctionType
F32 = mybir.dt.float32
BF16 = mybir.dt.bfloat16

N_CORES = 8
B = 65536
BC = B // N_CORES          # 8192 batch rows per core
D1 = 196                   # stage-1 nodes
D1E = 197                  # + bias row
D2 = 206                   # stage-2 nodes (+10 class)
D2E = 207
P = 128
D1B = D1 - P               # 68
D1KB = D1E - P             # 69
D2B = D2 - P               # 78
D2KB = D2E - P             # 79
NOUT = 10
BT = 512                   # batch tile (one PSUM bank of fp32)
CH = 1024                  # input/output DMA chunk (2 tiles)
SC = 1.1                   # sin argument scale (1 + eta)
DT = 0.5 / 5
DT2 = DT * DT
PI = float(np.pi)
# sin scale: a hair under pi so wrapped values at exactly +-1 stay inside
# the Act table's [-pi, pi] domain after the fp32 multiply
PI_SIN = float(np.float32(np.pi) * (1.0 - 3e-7))

TRACE = False
LAST_RESULTS = None

_CACHE = {}


# ---- custom DVE ops: +-1 bound / period-2 wrap in v = u/pi units ----------

def _wrap1(y):
    d = (y < (Zero - One)) - (One < y)
    return (y + d) + d


def _np_wrap(y):
    y = np.asarray(y, np.float32)
    return y + 2.0 * ((y < -1.0).astype(np.float32)
                      - (y > 1.0).astype(np.float32))


def _flat2(a, b):
    """CoreSim may pass the two operands with different (coalesced vs
    multi-dim) shapes; compare them [P, -1]."""
    a = np.asarray(a, np.float32)
    b = np.asarray(b, np.float32)
    return a.reshape(a.shape[0], -1), b.reshape(a.shape[0], -1)


ADD_WRAP_V = DveOp(
    "ADD_WRAP_V",
    Spec(
        body=_wrap1(Src0 + Src1),
        reference=lambda in0, in1, s0, s1, imm2: _np_wrap(
            sum(_flat2(in0, in1))),
    ),
    subdim=False,
    uops_sha={},
)

ADD2_WRAP_V = DveOp(
    "ADD2_WRAP_V",
    Spec(
        body=_wrap1((Src0 + Src0) + Src1),
        reference=lambda in0, in1, s0, s1, imm2: _np_wrap(
            (lambda a, b: 2.0 * a + b)(*_flat2(in0, in1))),
    ),
    subdim=False,
    uops_sha={},
)



def _register(op):
    if op.name in _SUB_OPCODE_FOR_NAME:
        return
    row = max(_SUB_OPCODE_FOR_NAME.values()) + 1
    assert row < 0x20, "custom-DVE row field is 5 bits"
    OPS.append(op)
    CUSTOM_DVE_SPECS[op.name] = op.spec
    _SUB_OPCODE_FOR_NAME[op.name] = row
    for ver in ("v3", "v4"):
        s = DveOpSpec(name=op.name, opcode=row,
                      uops=lower(op.spec, ver=ver), rd1_en=True)
        op.uops_sha[ver] = s.sha(ver)


_register(ADD_WRAP_V)
_register(ADD2_WRAP_V)


# ---- device program -------------------------------------------------------

def _build_program(bc=BC, num_devices=N_CORES, hoist=True):
    ch = min(CH, bc)
    nch = bc // ch             # input/output chunks
    tpc = ch // BT             # tiles per chunk
    nc = bacc.Bacc(
        "TRN2",
        target_bir_lowering=False,
        debug=False,
        num_devices=num_devices,
    )
    v0_d = nc.dram_tensor("v0", [P, 2, bc], F32, kind="ExternalInput").ap()
    s0_d = nc.dram_tensor("s0", [P, 2, bc], BF16, kind="ExternalInput").ap()
    w1_d = nc.dram_tensor("w1", [D1E, D1], BF16, kind="ExternalInput").ap()
    w2_d = nc.dram_tensor("w2", [D1E, D1], BF16, kind="ExternalInput").ap()
    w6_d = nc.dram_tensor("w6", [D1E, D1], BF16, kind="ExternalInput").ap()
    v1_d = nc.dram_tensor("v1", [D2E, D2E], BF16, kind="ExternalInput").ap()
    v2_d = nc.dram_tensor("v2", [D2E, D2E], BF16, kind="ExternalInput").ap()
    v6_d = nc.dram_tensor("v6", [D2E, D2E], BF16, kind="ExternalInput").ap()
    # output rows = stage-2 b-bank partitions 64:80 (class nodes at 69:79)
    out_d = nc.dram_tensor("out", [16, bc], F32, kind="ExternalOutput").ap()

    with tile.TileContext(nc) as tc:
        with (
            tc.tile_pool(name="wts", bufs=1) as wp,
            tc.tile_pool(name="io", bufs=3) as io,
            tc.tile_pool(name="sq", bufs=5) as sq,
            tc.tile_pool(name="psAB", bufs=2, space=bass.MemorySpace.PSUM) as psAB,
            tc.tile_pool(name="psCD", bufs=2, space=bass.MemorySpace.PSUM) as psCD,
        ):
            def load_w(dram, rows, name):
                ta = wp.tile([P, dram.shape[1]], BF16, tag=name + "a")
                nc.sync.dma_start(ta[:], dram[0:P, :])
                tb = wp.tile([rows - P, dram.shape[1]], BF16, tag=name + "b")
                nc.sync.dma_start(tb[:], dram[P:rows, :])
                return ta, tb

            def load_chunk(c):
                """Input chunk c: wide-packed [128, 2, ch] (slot 0 = state
                rows 0:128, slot 1 = rows 128:196 + ones row + zero pad)."""
                ccs = slice(c * ch, (c + 1) * ch)
                sm = io.tile([P, 2, ch], BF16, tag="sm")
                nc.sync.dma_start(sm[:], s0_d[:, :, ccs])
                vm = io.tile([P, 2, ch], F32, tag="vm")
                nc.sync.dma_start(vm[:], v0_d[:, :, ccs])
                return vm, sm

            # chunk-0 inputs first so the first tile isn't stuck behind
            # 1MB of weight DMAs on the single HWDGE
            nxt = load_chunk(0)
            w1a, w1b = load_w(w1_d, D1E, "w1")
            v1a, v1b = load_w(v1_d, D2E, "v1")
            w2a, w2b = load_w(w2_d, D1E, "w2")
            w6a, w6b = load_w(w6_d, D1E, "w6")
            v2a, v2b = load_w(v2_d, D2E, "v2")
            v6a, v6b = load_w(v6_d, D2E, "v6")

            def wsin(src, name):
                """One wide sin over a [128, 2, BT] wrapped-state tile.
                Constant rows self-maintain: the 0.5 driver rows become
                sin(pi/2) = 1 (the folded-bias ones rows), zero pad rows
                become 0."""
                t = sq.tile([P, 2, BT], BF16, tag=name)
                nc.scalar.activation(t[:], src[:], AF.Sin, scale=PI_SIN)
                return t

            def wwrap(op, bank, base, name):
                """One wide [128, 2, BT] custom-DVE op over a merged
                2-bank PSUM tile; slot-1 rows past the live region are
                zeros (one-time bank init + host zero padding)."""
                w = sq.tile([P, 2, BT], F32, tag="wr" + name)
                nc.vector._custom_dve(op, out=w[:], in0=bank[:],
                                      in1=base, s0=0.0, s1=0.0)
                return w

            def emit_st1(t01, chunk, fresh, fresh_ps):
                vm, sm = chunk
                cs = slice((t01 % tpc) * BT, (t01 % tpc + 1) * BT)
                xm = vm[:, :, cs]
                s0a, s0b = sm[:, 0, cs], sm[0:D1KB, 1, cs]
                pab = psAB.tile([P, 2, BT], F32, tag="AB")
                if fresh_ps:
                    # matmuls never touch slot-1 rows 68:128; zero them so
                    # the wide wraps see finite values (and the class rows
                    # of z2 come out exactly zero)
                    nc.scalar.memzero(pab[64:P, 1, :])
                pa, pb = pab[:, 0, :], pab[0:D1B, 1, :]

                def s1_pass(wta, wtb, ra, rb, start=False, stop=False):
                    nc.tensor.matmul(pa, wta[:, 0:P], ra,
                                     start=start, stop=stop,
                                     skip_group_check=True)
                    nc.tensor.matmul(pa, wtb[:, 0:P], rb,
                                     start=False, stop=stop,
                                     skip_group_check=True)
                    nc.tensor.matmul(pb, wta[:, P:D1], ra,
                                     start=start, stop=stop,
                                     skip_group_check=True)
                    nc.tensor.matmul(pb, wtb[:, P:D1], rb,
                                     start=False, stop=stop,
                                     skip_group_check=True)

                s1_pass(w1a, w1b, s0a, s0b, start=True)      # g0 + e
                u2 = wwrap(ADD_WRAP_V, pab, xm, "2")
                s2 = wsin(u2, "s2")
                s2a, s2b = s2[:, 0, :], s2[0:D1KB, 1, :]
                u3 = wwrap(ADD2_WRAP_V, pab, u2[:], "3")
                s3 = wsin(u3, "s3")
                s3a, s3b = s3[:, 0, :], s3[0:D1KB, 1, :]
                s1_pass(w6a, w6b, s0a, s0b)                  # -> 7g0, 7e
                s1_pass(w2a, w2b, s2a, s2b)                  # + 2g2, 9e
                s1_pass(w1a, w1b, s3a, s3b, stop=True)       # + g3, 10e
                # z2 = wrap(v0 + bank); slot-1 rows 68:78 are exactly the
                # class zeros (0 + 0)
                return wwrap(ADD_WRAP_V, pab, xm, "z")

            def emit_st2(t01, z2, outst, fresh, fresh_ps):
                cs = slice((t01 % tpc) * BT, (t01 % tpc + 1) * BT)
                t0 = wsin(z2, "t0")
                t0a, t0b = t0[:, 0, :], t0[0:D2KB, 1, :]
                yab = psCD.tile([P, 2, BT], F32, tag="CD")
                if fresh_ps:
                    nc.scalar.memzero(yab[64:P, 1, :])
                ya, yb = yab[:, 0, :], yab[0:D2KB, 1, :]

                def s2_trim(wta, wtb, ra, rb, stop=False):
                    nc.tensor.matmul(yb, wta[:, P:D2E], ra,
                                     start=False, stop=stop,
                                     skip_group_check=True)
                    nc.tensor.matmul(yb, wtb[:, P:D2E], rb,
                                     start=False, stop=stop,
                                     skip_group_check=True)

                # g0' + e': ya's group opens and closes here (it is only
                # read afterwards); yb keeps accumulating
                nc.tensor.matmul(ya, v1a[:, 0:P], t0a,
                                 start=True, stop=False,
                                 skip_group_check=True)
                nc.tensor.matmul(ya, v1b[:, 0:P], t0b,
                                 start=False, stop=True,
                                 skip_group_check=True)
                nc.tensor.matmul(yb, v1a[:, P:D2E], t0a,
                                 start=True, stop=False,
                                 skip_group_check=True)
                nc.tensor.matmul(yb, v1b[:, P:D2E], t0b,
                                 start=False, stop=False,
                                 skip_group_check=True)
                n2 = wwrap(ADD_WRAP_V, yab, z2[:], "2p")
                t2 = wsin(n2, "t2")
                t2a, t2b = t2[:, 0, :], t2[0:D2KB, 1, :]
                n3 = wwrap(ADD2_WRAP_V, yab, n2[:], "3p")
                t3 = wsin(n3, "t3")
                t3a, t3b = t3[:, 0, :], t3[0:D2KB, 1, :]
                s2_trim(v6a, v6b, t0a, t0b)                  # -> 7g0'
                s2_trim(v2a, v2b, t2a, t2b)                  # + 2g2'
                s2_trim(v1a, v1b, t3a, t3b, stop=True)       # + g3'
                nc.scalar.copy(outst[64:80, cs], yab[64:80, 1, :])

            # software-pipelined emission: stage-1 of tile t runs ahead of
            # stage-2 of tile t-1 so each engine's in-order queue interleaves
            # the two stages instead of head-of-line blocking on tile t-1's
            # tail ops
            ntiles = nch * tpc
            chunks = {0: nxt}
            outsts = {}
            pend = {}
            for t in range(ntiles + 2):
                if t >= 2:
                    tp = t - 2
                    cp = tp // tpc
                    if tp % tpc == 0:
                        outsts[cp] = io.tile([80, ch], F32, tag="outst",
                                             name=f"outst{cp}")
                if t < ntiles:
                    c = t // tpc
                    if t % tpc == 0 and c + 1 < nch:
                        chunks[c + 1] = load_chunk(c + 1)
                    fresh1 = (not hoist) or t < 4
                    fps1 = (not hoist) or t < 2
                    pend[t] = emit_st1(t, chunks[c], fresh1, fps1)
                if t >= 2:
                    tp = t - 2
                    cp = tp // tpc
                    fresh2 = (not hoist) or tp < 4
                    fps2 = (not hoist) or tp < 2
                    emit_st2(tp, pend.pop(tp), outsts[cp], fresh2, fps2)
                    if tp % tpc == tpc - 1:
                        ccs = slice(cp * ch, (cp + 1) * ch)
                        nc.sync.dma_start(out_d[:, ccs],
                                          outsts.pop(cp)[64:80, :])

    nc.compile()
    return nc


# ---- host side ------------------------------------------------------------

def _c2q(C):
    Q = 0.5 * (C + C.T)
    d = -Q.sum(axis=0)
    Q = Q.copy()
    Q[np.diag_indices_from(Q)] = d
    return Q


def _permute_v(v):
    """Stage-2 weight layout for the wide-packed state: contraction rows
    [state 0:196 | e | class 196:206] (the e-row rides the 0.5-driver at
    slot-1 row 68), target cols [0:196 | dummy zero | class] so the
    class targets land at slot-1 partitions 69:79."""
    rows = np.concatenate([v[0:D2 - NOUT], v[D2:D2 + 1],
                           v[D2 - NOUT:D2]], axis=0)
    return np.insert(rows, D2 - NOUT, 0.0, axis=1)


def _host_weights(fc_w, fc_b, qn, dim):
    """v-units dynamics matrix (scaled by 1.1*DT^2/pi), bias as last row."""
    W = (SC * DT2 / PI) * (_c2q(np.asarray(fc_w, np.float64))
                           + np.asarray(qn, np.float64) - np.eye(dim))
    eb = (SC * DT2 / PI) * np.asarray(fc_b, np.float64)
    return np.concatenate([W, eb[None, :]], axis=0)


def kernel(x, fc1_w, fc1_b, fc2_w, fc2_b, output_fac,
           Q_noise_small, Q_noise_large):
    global LAST_RESULTS
    if "nc" not in _CACHE:
        _CACHE["nc"] = _build_program()
    nc = _CACHE["nc"]

    w1 = _host_weights(fc1_w, fc1_b, Q_noise_small, D1)
    v1 = _host_weights(fc2_w, fc2_b, Q_noise_large, D2)
    v1 = _permute_v(v1)

    BF = ml_dtypes.bfloat16

    def bf(a):
        return np.ascontiguousarray(np.asarray(a, np.float32).astype(BF))

    # v0 = 1.1*x/pi wrapped into [-1, 1]; s0 = sin(1.1 x) exactly.  Both
    # ship wide-packed [128, 2, B]: slot 0 = state rows 0:128, slot 1 =
    # rows 128:196 (+ a ones row in s0 for the folded e-bias, zero pad)
    u = (SC / PI) * np.asarray(x, np.float64)
    u = u - 2.0 * ((u > 1.0).astype(np.float64) - (u < -1.0).astype(np.float64))
    vt = np.asarray(u.T, np.float32)                   # [D1, B]
    st = np.sin(PI * np.asarray(u.T, np.float64)).astype(np.float32)
    nb = vt.shape[1]
    vm = np.zeros((P, 2, nb), np.float32)
    vm[:, 0, :] = vt[0:P]
    vm[0:D1B, 1, :] = vt[P:D1]
    vm[D1B, 1, :] = 0.5          # ones-driver: every wrap keeps it 0.5,
    sm = np.zeros((P, 2, nb), np.float32)   # every sin turns it into 1.0
    sm[:, 0, :] = st[0:P]
    sm[0:D1B, 1, :] = st[P:D1]
    sm[D1B, 1, :] = 1.0

    common = {
        "w1": bf(w1), "w2": bf(2.0 * w1), "w6": bf(6.0 * w1),
        "v1": bf(v1), "v2": bf(2.0 * v1), "v6": bf(6.0 * v1),
    }
    in_maps = []
    for c in range(N_CORES):
        m = dict(common)
        m["v0"] = np.ascontiguousarray(vm[:, :, c * BC:(c + 1) * BC])
        m["s0"] = np.ascontiguousarray(sm[:, :, c * BC:(c + 1) * BC]).astype(BF)
        in_maps.append(m)

    res = None
    last_exc = None
    for _attempt in range(3):
        try:
            res = run_bass_kernel_spmd(
                nc, in_maps, core_ids=list(range(N_CORES)), trace=TRACE)
            break
        except Exception as e:  # transient NRT/device hiccups
            last_exc = e
            try:
                import time as _time

                import jax as _jax
                _jax.clear_caches()
                if hasattr(_jax, "clear_backends"):
                    _jax.clear_backends()
                _time.sleep(5)
            except Exception:
                pass
    if res is None:
        raise last_exc
    LAST_RESULTS = res

    out = np.empty((B, NOUT), np.float32)
    for c in range(N_CORES):
        out[c * BC:(c + 1) * BC, :] = res.results[c]["out"][5:15, :].T
    fac = float(np.asarray(output_fac)) * PI / SC
    return out * np.float32(fac)


# revision 44
# speedup vs baseline: 1.0033x; 1.0033x over previous
"""Trainium2 Bass kernel for nn_Net_75282186764473.

Math: reference pat() returns zm + stop_gradient(ze - zm) == ze numerically;
the forward pass is 5 explicit-Euler steps of the 'experiment' dynamics per
stage, twice:  q' = p ; p' = sin(1.1 q) @ (c2q(C) + Qn - I) + e.
With u = 1.1 q, g_n = sin(u_n) @ W + eb  (W, eb scaled by 1.1*DT^2):
    u2 = u0 + g0 ; u3 = u0 + 3 g0 ; u5 = u0 + 7 g0 + 2 g2 + g3
so each stage needs sins at u0, u2, u3 and weighted passes {1,6,2,1}*W:
the 6W pass tops the g0 PSUM bank up to 7g0 after its last read, then the
2W/1W passes accumulate 2g2 + g3 into the same bank.

All state lives in v = u/pi units so the range wrap is 'bound +-1,
period 2', which two custom DVE ops implement with zero constant slots:
    ADD_WRAP_V  : out = wrap(in0 + in1)
    ADD2_WRAP_V : out = wrap(2*in0 + in1)
(u3 = wrap(u2_wrapped + 2*g0) is exact mod 2.)  sin(u) = Act Sin with
scale=pi on the v-state, so the table sin stays exact; weights carry the
1/pi.  e-biases ride as an extra contraction row driven by a constant
0.5 in the wide state (every wrap maps 0.5 -> 0.5, every sin -> 1.0).

Layout: everything is wide-packed [128, 2, 512] per batch tile -- slot 0
= node rows 0:128, slot 1 = rows 128:196 + driver/class/pad rows -- so
each wrap is ONE wide DVE op over a merged 2-bank PSUM tile and each sin
is ONE wide Act op.  Stage 2 permutes the weights (_permute_v) so its
slot-1 contraction is [state 68 | e-row | class 10] and its targets are
[state 68 | dummy | class 10]: the class-node zeros fall out of zeroed
bank rows + zero input padding automatically.  Per tile: 26 matmuls
(512-col, bf16), 5 wide wraps (DVE), 5 wide sins + 1 copy (Act).
sin(1.1 x) is precomputed on the host and shipped as bf16 next to the
fp32 v0 state.  Stage-1 emission runs two tiles ahead of stage-2
(software pipelining) so the in-order engine queues stay fed.

Sharding: pure batch data-parallel across 8 cores (8192 rows each); the
outputs are PSUM slot-1 partitions 69:79 of the stage-2 bank, scaled by
pi/1.1 on the host.
"""

import ml_dtypes
import numpy as np

import concourse.bacc as bacc
import concourse.bass as bass
import concourse.mybir as mybir
import concourse.tile as tile
from concourse.bass_utils import run_bass_kernel_spmd
from concourse.dve_ops import (
    CUSTOM_DVE_SPECS,
    OPS,
    DveOp,
    _SUB_OPCODE_FOR_NAME,
)
from concourse.dve_spec import One, Spec, Src0, Src1, Zero, lower
from concourse.dve_uop import DveOpSpec

AF = mybir.ActivationFunctionType
F32 = mybir.dt.float32
BF16 = mybir.dt.bfloat16

N_CORES = 8
B = 65536
BC = B // N_CORES          # 8192 batch rows per core
D1 = 196                   # stage-1 nodes
D1E = 197                  # + bias row
D2 = 206                   # stage-2 nodes (+10 class)
D2E = 207
P = 128
D1B = D1 - P               # 68
D1KB = D1E - P             # 69
D2B = D2 - P               # 78
D2KB = D2E - P             # 79
NOUT = 10
BT = 512                   # batch tile (one PSUM bank of fp32)
CH = 1024                  # input/output DMA chunk (2 tiles)
SC = 1.1                   # sin argument scale (1 + eta)
DT = 0.5 / 5
DT2 = DT * DT
PI = float(np.pi)
# sin scale: a hair under pi so wrapped values at exactly +-1 stay inside
# the Act table's [-pi, pi] domain after the fp32 multiply
PI_SIN = float(np.float32(np.pi) * (1.0 - 3e-7))

TRACE = False
LAST_RESULTS = None

_CACHE = {}


# ---- custom DVE ops: +-1 bound / period-2 wrap in v = u/pi units ----------

def _wrap1(y):
    d = (y < (Zero - One)) - (One < y)
    return (y + d) + d


def _np_wrap(y):
    y = np.asarray(y, np.float32)
    return y + 2.0 * ((y < -1.0).astype(np.float32)
                      - (y > 1.0).astype(np.float32))


def _flat2(a, b):
    """CoreSim may pass the two operands with different (coalesced vs
    multi-dim) shapes; compare them [P, -1]."""
    a = np.asarray(a, np.float32)
    b = np.asarray(b, np.float32)
    return a.reshape(a.shape[0], -1), b.reshape(a.shape[0], -1)


ADD_WRAP_V = DveOp(
    "ADD_WRAP_V",
    Spec(
        body=_wrap1(Src0 + Src1),
        reference=lambda in0, in1, s0, s1, imm2: _np_wrap(
            sum(_flat2(in0, in1))),
    ),
    subdim=False,
    uops_sha={},
)

ADD2_WRAP_V = DveOp(
    "ADD2_WRAP_V",
    Spec(
        body=_wrap1((Src0 + Src0) + Src1),
        reference=lambda in0, in1, s0, s1, imm2: _np_wrap(
            (lambda a, b: 2.0 * a + b)(*_flat2(in0, in1))),
    ),
    subdim=False,
    uops_sha={},
)



def _register(op):
    if op.name in _SUB_OPCODE_FOR_NAME:
        return
    row = max(_SUB_OPCODE_FOR_NAME.values()) + 1
    assert row < 0x20, "custom-DVE row field is 5 bits"
    OPS.append(op)
    CUSTOM_DVE_SPECS[op.name] = op.spec
    _SUB_OPCODE_FOR_NAME[op.name] = row
    for ver in ("v3", "v4"):
        s = DveOpSpec(name=op.name, opcode=row,
                      uops=lower(op.spec, ver=ver), rd1_en=True)
        op.uops_sha[ver] = s.sha(ver)


_register(ADD_WRAP_V)
_register(ADD2_WRAP_V)


# ---- device program -------------------------------------------------------

def _build_program(bc=BC, num_devices=N_CORES, hoist=True):
    ch = min(CH, bc)
    nch = bc // ch             # input/output chunks
    tpc = ch // BT             # tiles per chunk
    nc = bacc.Bacc(
        "TRN2",
        target_bir_lowering=False,
        debug=False,
        num_devices=num_devices,
    )
    v0_d = nc.dram_tensor("v0", [P, 2, bc], F32, kind="ExternalInput").ap()
    s0_d = nc.dram_tensor("s0", [P, 2, bc], BF16, kind="ExternalInput").ap()
    w1_d = nc.dram_tensor("w1", [D1E, D1], BF16, kind="ExternalInput").ap()
    w2_d = nc.dram_tensor("w2", [D1E, D1], BF16, kind="ExternalInput").ap()
    w6_d = nc.dram_tensor("w6", [D1E, D1], BF16, kind="ExternalInput").ap()
    v1_d = nc.dram_tensor("v1", [D2E, D2E], BF16, kind="ExternalInput").ap()
    v2_d = nc.dram_tensor("v2", [D2E, D2E], BF16, kind="ExternalInput").ap()
    v6_d = nc.dram_tensor("v6", [D2E, D2E], BF16, kind="ExternalInput").ap()
    # output rows = stage-2 b-bank partitions 64:80 (class nodes at 69:79)
    out_d = nc.dram_tensor("out", [16, bc], F32, kind="ExternalOutput").ap()

    with tile.TileContext(nc) as tc:
        with (
            tc.tile_pool(name="wts", bufs=1) as wp,
            tc.tile_pool(name="io", bufs=3) as io,
            tc.tile_pool(name="sq", bufs=5) as sq,
            tc.tile_pool(name="psAB", bufs=2, space=bass.MemorySpace.PSUM) as psAB,
            tc.tile_pool(name="psCD", bufs=2, space=bass.MemorySpace.PSUM) as psCD,
        ):
            def load_w(dram, rows, name):
                ta = wp.tile([P, dram.shape[1]], BF16, tag=name + "a")
                nc.sync.dma_start(ta[:], dram[0:P, :])
                tb = wp.tile([rows - P, dram.shape[1]], BF16, tag=name + "b")
                nc.sync.dma_start(tb[:], dram[P:rows, :])
                return ta, tb

            def load_chunk(c, mid=None):
                """Input chunk c: wide-packed [128, 2, ch] (slot 0 = state
                rows 0:128, slot 1 = rows 128:196 + ones row + zero pad).
                `mid` runs between the two DMAs (chunk 0 slots the w1
                load there: the first matmul needs s0+w1, the first wrap
                needs v0 only later)."""
                ccs = slice(c * ch, (c + 1) * ch)
                sm = io.tile([P, 2, ch], BF16, tag="sm")
                nc.sync.dma_start(sm[:], s0_d[:, :, ccs])
                r = mid() if mid else None
                vm = io.tile([P, 2, ch], F32, tag="vm")
                nc.sync.dma_start(vm[:], v0_d[:, :, ccs])
                return (vm, sm) if mid is None else (vm, sm, r)

            vm0, sm0, (w1a, w1b) = load_chunk(
                0, mid=lambda: load_w(w1_d, D1E, "w1"))
            nxt = (vm0, sm0)
            v1a, v1b = load_w(v1_d, D2E, "v1")
            w2a, w2b = load_w(w2_d, D1E, "w2")
            w6a, w6b = load_w(w6_d, D1E, "w6")
            v2a, v2b = load_w(v2_d, D2E, "v2")
            v6a, v6b = load_w(v6_d, D2E, "v6")

            def wsin(src, name):
                """One wide sin over a [128, 2, BT] wrapped-state tile.
                Constant rows self-maintain: the 0.5 driver rows become
                sin(pi/2) = 1 (the folded-bias ones rows), zero pad rows
                become 0."""
                t = sq.tile([P, 2, BT], BF16, tag=name)
                nc.scalar.activation(t[:], src[:], AF.Sin, scale=PI_SIN)
                return t

            def wwrap(op, bank, base, name):
                """One wide [128, 2, BT] custom-DVE op over a merged
                2-bank PSUM tile; slot-1 rows past the live region are
                zeros (one-time bank init + host zero padding)."""
                w = sq.tile([P, 2, BT], F32, tag="wr" + name)
                nc.vector._custom_dve(op, out=w[:], in0=bank[:],
                                      in1=base, s0=0.0, s1=0.0)
                return w

            def emit_st1(t01, chunk, fresh, fresh_ps):
                vm, sm = chunk
                cs = slice((t01 % tpc) * BT, (t01 % tpc + 1) * BT)
                xm = vm[:, :, cs]
                s0a, s0b = sm[:, 0, cs], sm[0:D1KB, 1, cs]
                pab = psAB.tile([P, 2, BT], F32, tag="AB")
                if fresh_ps:
                    # matmuls never touch slot-1 rows 68:128; zero them so
                    # the wide wraps see finite values (and the class rows
                    # of z2 come out exactly zero)
                    nc.scalar.memzero(pab[64:P, 1, :])
                pa, pb = pab[:, 0, :], pab[0:D1B, 1, :]

                def s1_pass(wta, wtb, ra, rb, start=False, stop=False):
                    nc.tensor.matmul(pa, wta[:, 0:P], ra,
                                     start=start, stop=stop,
                                     skip_group_check=True)
                    nc.tensor.matmul(pa, wtb[:, 0:P], rb,
                                     start=False, stop=stop,
                                     skip_group_check=True)
                    nc.tensor.matmul(pb, wta[:, P:D1], ra,
                                     start=start, stop=stop,
                                     skip_group_check=True)
                    nc.tensor.matmul(pb, wtb[:, P:D1], rb,
                                     start=False, stop=stop,
                                     skip_group_check=True)

                s1_pass(w1a, w1b, s0a, s0b, start=True)      # g0 + e
                u2 = wwrap(ADD_WRAP_V, pab, xm, "2")
                s2 = wsin(u2, "s2")
                s2a, s2b = s2[:, 0, :], s2[0:D1KB, 1, :]
                u3 = wwrap(ADD2_WRAP_V, pab, u2[:], "3")
                s3 = wsin(u3, "s3")
                s3a, s3b = s3[:, 0, :], s3[0:D1KB, 1, :]
                s1_pass(w6a, w6b, s0a, s0b)                  # -> 7g0, 7e
                s1_pass(w2a, w2b, s2a, s2b)                  # + 2g2, 9e
                s1_pass(w1a, w1b, s3a, s3b, stop=True)       # + g3, 10e
                # z2 = wrap(v0 + bank); slot-1 rows 68:78 are exactly the
                # class zeros (0 + 0)
                return wwrap(ADD_WRAP_V, pab, xm, "z")

            def emit_st2(t01, z2, outst, fresh, fresh_ps):
                cs = slice((t01 % tpc) * BT, (t01 % tpc + 1) * BT)
                t0 = wsin(z2, "t0")
                t0a, t0b = t0[:, 0, :], t0[0:D2KB, 1, :]
                yab = psCD.tile([P, 2, BT], F32, tag="CD")
                if fresh_ps:
                    nc.scalar.memzero(yab[64:P, 1, :])
                ya, yb = yab[:, 0, :], yab[0:D2KB, 1, :]

                def s2_trim(wta, wtb, ra, rb, stop=False):
                    nc.tensor.matmul(yb, wta[:, P:D2E], ra,
                                     start=False, stop=stop,
                                     skip_group_check=True)
                    nc.tensor.matmul(yb, wtb[:, P:D2E], rb,
                                     start=False, stop=stop,
                                     skip_group_check=True)

                # g0' + e': ya's group opens and closes here (it is only
                # read afterwards); yb keeps accumulating
                nc.tensor.matmul(ya, v1a[:, 0:P], t0a,
                                 start=True, stop=False,
                                 skip_group_check=True)
                nc.tensor.matmul(ya, v1b[:, 0:P], t0b,
                                 start=False, stop=True,
                                 skip_group_check=True)
                nc.tensor.matmul(yb, v1a[:, P:D2E], t0a,
                                 start=True, stop=False,
                                 skip_group_check=True)
                nc.tensor.matmul(yb, v1b[:, P:D2E], t0b,
                                 start=False, stop=False,
                                 skip_group_check=True)
                n2 = wwrap(ADD_WRAP_V, yab, z2[:], "2p")
                t2 = wsin(n2, "t2")
                t2a, t2b = t2[:, 0, :], t2[0:D2KB, 1, :]
                n3 = wwrap(ADD2_WRAP_V, yab, n2[:], "3p")
                t3 = wsin(n3, "t3")
                t3a, t3b = t3[:, 0, :], t3[0:D2KB, 1, :]
                s2_trim(v6a, v6b, t0a, t0b)                  # -> 7g0'
                s2_trim(v2a, v2b, t2a, t2b)                  # + 2g2'
                s2_trim(v1a, v1b, t3a, t3b, stop=True)       # + g3'
                nc.scalar.copy(outst[64:80, cs], yab[64:80, 1, :])

            # software-pipelined emission: stage-1 of tile t runs ahead of
            # stage-2 of tile t-1 so each engine's in-order queue interleaves
            # the two stages instead of head-of-line blocking on tile t-1's
            # tail ops
            ntiles = nch * tpc
            chunks = {0: nxt}
            outsts = {}
            pend = {}
            for t in range(ntiles + 2):
                if t >= 2:
                    tp = t - 2
                    cp = tp // tpc
                    if tp % tpc == 0:
                        outsts[cp] = io.tile([80, ch], F32, tag="outst",
                                             name=f"outst{cp}")
                if t < ntiles:
                    c = t // tpc
                    if t % tpc == 0 and c + 1 < nch:
                        chunks[c + 1] = load_chunk(c + 1)
                    fresh1 = (not hoist) or t < 4
                    fps1 = (not hoist) or t < 2
                    pend[t] = emit_st1(t, chunks[c], fresh1, fps1)
                if t >= 2:
                    tp = t - 2
                    cp = tp // tpc
                    fresh2 = (not hoist) or tp < 4
                    fps2 = (not hoist) or tp < 2
                    emit_st2(tp, pend.pop(tp), outsts[cp], fresh2, fps2)
                    if tp % tpc == tpc - 1:
                        ccs = slice(cp * ch, (cp + 1) * ch)
                        nc.sync.dma_start(out_d[:, ccs],
                                          outsts.pop(cp)[64:80, :])

    nc.compile()
    return nc


# ---- host side ------------------------------------------------------------

def _c2q(C):
    Q = 0.5 * (C + C.T)
    d = -Q.sum(axis=0)
    Q = Q.copy()
    Q[np.diag_indices_from(Q)] = d
    return Q


def _permute_v(v):
    """Stage-2 weight layout for the wide-packed state: contraction rows
    [state 0:196 | e | class 196:206] (the e-row rides the 0.5-driver at
    slot-1 row 68), target cols [0:196 | dummy zero | class] so the
    class targets land at slot-1 partitions 69:79."""
    rows = np.concatenate([v[0:D2 - NOUT], v[D2:D2 + 1],
                           v[D2 - NOUT:D2]], axis=0)
    return np.insert(rows, D2 - NOUT, 0.0, axis=1)


def _host_weights(fc_w, fc_b, qn, dim):
    """v-units dynamics matrix (scaled by 1.1*DT^2/pi), bias as last row."""
    W = (SC * DT2 / PI) * (_c2q(np.asarray(fc_w, np.float64))
                           + np.asarray(qn, np.float64) - np.eye(dim))
    eb = (SC * DT2 / PI) * np.asarray(fc_b, np.float64)
    return np.concatenate([W, eb[None, :]], axis=0)


def kernel(x, fc1_w, fc1_b, fc2_w, fc2_b, output_fac,
           Q_noise_small, Q_noise_large):
    global LAST_RESULTS
    if "nc" not in _CACHE:
        _CACHE["nc"] = _build_program()
    nc = _CACHE["nc"]

    w1 = _host_weights(fc1_w, fc1_b, Q_noise_small, D1)
    v1 = _host_weights(fc2_w, fc2_b, Q_noise_large, D2)
    v1 = _permute_v(v1)

    BF = ml_dtypes.bfloat16

    def bf(a):
        return np.ascontiguousarray(np.asarray(a, np.float32).astype(BF))

    # v0 = 1.1*x/pi wrapped into [-1, 1]; s0 = sin(1.1 x) exactly.  Both
    # ship wide-packed [128, 2, B]: slot 0 = state rows 0:128, slot 1 =
    # rows 128:196 (+ a ones row in s0 for the folded e-bias, zero pad)
    u = (SC / PI) * np.asarray(x, np.float64)
    u = u - 2.0 * ((u > 1.0).astype(np.float64) - (u < -1.0).astype(np.float64))
    vt = np.asarray(u.T, np.float32)                   # [D1, B]
    st = np.sin(PI * np.asarray(u.T, np.float64)).astype(np.float32)
    nb = vt.shape[1]
    vm = np.zeros((P, 2, nb), np.float32)
    vm[:, 0, :] = vt[0:P]
    vm[0:D1B, 1, :] = vt[P:D1]
    vm[D1B, 1, :] = 0.5          # ones-driver: every wrap keeps it 0.5,
    sm = np.zeros((P, 2, nb), np.float32)   # every sin turns it into 1.0
    sm[:, 0, :] = st[0:P]
    sm[0:D1B, 1, :] = st[P:D1]
    sm[D1B, 1, :] = 1.0

    common = {
        "w1": bf(w1), "w2": bf(2.0 * w1), "w6": bf(6.0 * w1),
        "v1": bf(v1), "v2": bf(2.0 * v1), "v6": bf(6.0 * v1),
    }
    in_maps = []
    for c in range(N_CORES):
        m = dict(common)
        m["v0"] = np.ascontiguousarray(vm[:, :, c * BC:(c + 1) * BC])
        m["s0"] = np.ascontiguousarray(sm[:, :, c * BC:(c + 1) * BC]).astype(BF)
        in_maps.append(m)

    res = None
    last_exc = None
    for _attempt in range(3):
        try:
            res = run_bass_kernel_spmd(
                nc, in_maps, core_ids=list(range(N_CORES)), trace=TRACE)
            break
        except Exception as e:  # transient NRT/device hiccups
            last_exc = e
            try:
                import time as _time

                import jax as _jax
                _jax.clear_caches()
                if hasattr(_jax, "clear_backends"):
                    _jax.clear_backends()
                _time.sleep(5)
            except Exception:
                pass
    if res is None:
        raise last_exc
    LAST_RESULTS = res

    out = np.empty((B, NOUT), np.float32)
    for c in range(N_CORES):
        out[c * BC:(c + 1) * BC, :] = res.results[c]["out"][5:15, :].T
    fac = float(np.asarray(output_fac)) * PI / SC
    return out * np.float32(fac)


# revision 45
# speedup vs baseline: 1.0041x; 1.0008x over previous
"""Trainium2 Bass kernel for nn_Net_75282186764473.

Math: reference pat() returns zm + stop_gradient(ze - zm) == ze numerically;
the forward pass is 5 explicit-Euler steps of the 'experiment' dynamics per
stage, twice:  q' = p ; p' = sin(1.1 q) @ (c2q(C) + Qn - I) + e.
With u = 1.1 q, g_n = sin(u_n) @ W + eb  (W, eb scaled by 1.1*DT^2):
    u2 = u0 + g0 ; u3 = u0 + 3 g0 ; u5 = u0 + 7 g0 + 2 g2 + g3
so each stage needs sins at u0, u2, u3 and weighted passes {1,6,2,1}*W:
the 6W pass tops the g0 PSUM bank up to 7g0 after its last read, then the
2W/1W passes accumulate 2g2 + g3 into the same bank.

All state lives in v = u/pi units so the range wrap is 'bound +-1,
period 2', which two custom DVE ops implement with zero constant slots:
    ADD_WRAP_V  : out = wrap(in0 + in1)
    ADD2_WRAP_V : out = wrap(2*in0 + in1)
(u3 = wrap(u2_wrapped + 2*g0) is exact mod 2.)  sin(u) = Act Sin with
scale=pi on the v-state, so the table sin stays exact; weights carry the
1/pi.  e-biases ride as an extra contraction row driven by a constant
0.5 in the wide state (every wrap maps 0.5 -> 0.5, every sin -> 1.0).

Layout: everything is wide-packed [128, 2, 512] per batch tile -- slot 0
= node rows 0:128, slot 1 = rows 128:196 + driver/class/pad rows -- so
each wrap is ONE wide DVE op over a merged 2-bank PSUM tile and each sin
is ONE wide Act op.  Stage 2 permutes the weights (_permute_v) so its
slot-1 contraction is [state 68 | e-row | class 10] and its targets are
[state 68 | dummy | class 10]: the class-node zeros fall out of zeroed
bank rows + zero input padding automatically.  Per tile: 26 matmuls
(512-col, bf16), 5 wide wraps (DVE), 5 wide sins + 1 copy (Act).
sin(1.1 x) is precomputed on the host and shipped as bf16 next to the
fp32 v0 state.  Stage-1 emission runs two tiles ahead of stage-2
(software pipelining) so the in-order engine queues stay fed.

Sharding: pure batch data-parallel across 8 cores (8192 rows each); the
outputs are PSUM slot-1 partitions 69:79 of the stage-2 bank, scaled by
pi/1.1 on the host.
"""

import ml_dtypes
import numpy as np

import concourse.bacc as bacc
import concourse.bass as bass
import concourse.mybir as mybir
import concourse.tile as tile
from concourse.bass_utils import run_bass_kernel_spmd
from concourse.dve_ops import (
    CUSTOM_DVE_SPECS,
    OPS,
    DveOp,
    _SUB_OPCODE_FOR_NAME,
)
from concourse.dve_spec import One, Spec, Src0, Src1, Zero, lower
from concourse.dve_uop import DveOpSpec

AF = mybir.ActivationFunctionType
F32 = mybir.dt.float32
BF16 = mybir.dt.bfloat16

N_CORES = 8
B = 65536
BC = B // N_CORES          # 8192 batch rows per core
D1 = 196                   # stage-1 nodes
D1E = 197                  # + bias row
D2 = 206                   # stage-2 nodes (+10 class)
D2E = 207
P = 128
D1B = D1 - P               # 68
D1KB = D1E - P             # 69
D2B = D2 - P               # 78
D2KB = D2E - P             # 79
NOUT = 10
BT = 512                   # batch tile (one PSUM bank of fp32)
CH = 1024                  # input/output DMA chunk (2 tiles)
SC = 1.1                   # sin argument scale (1 + eta)
DT = 0.5 / 5
DT2 = DT * DT
PI = float(np.pi)
# sin scale: a hair under pi so wrapped values at exactly +-1 stay inside
# the Act table's [-pi, pi] domain after the fp32 multiply
PI_SIN = float(np.float32(np.pi) * (1.0 - 3e-7))

TRACE = False
LAST_RESULTS = None

_CACHE = {}


# ---- custom DVE ops: +-1 bound / period-2 wrap in v = u/pi units ----------

def _wrap1(y):
    d = (y < (Zero - One)) - (One < y)
    return (y + d) + d


def _np_wrap(y):
    y = np.asarray(y, np.float32)
    return y + 2.0 * ((y < -1.0).astype(np.float32)
                      - (y > 1.0).astype(np.float32))


def _flat2(a, b):
    """CoreSim may pass the two operands with different (coalesced vs
    multi-dim) shapes; compare them [P, -1]."""
    a = np.asarray(a, np.float32)
    b = np.asarray(b, np.float32)
    return a.reshape(a.shape[0], -1), b.reshape(a.shape[0], -1)


ADD_WRAP_V = DveOp(
    "ADD_WRAP_V",
    Spec(
        body=_wrap1(Src0 + Src1),
        reference=lambda in0, in1, s0, s1, imm2: _np_wrap(
            sum(_flat2(in0, in1))),
    ),
    subdim=False,
    uops_sha={},
)

ADD2_WRAP_V = DveOp(
    "ADD2_WRAP_V",
    Spec(
        body=_wrap1((Src0 + Src0) + Src1),
        reference=lambda in0, in1, s0, s1, imm2: _np_wrap(
            (lambda a, b: 2.0 * a + b)(*_flat2(in0, in1))),
    ),
    subdim=False,
    uops_sha={},
)



def _register(op):
    if op.name in _SUB_OPCODE_FOR_NAME:
        return
    row = max(_SUB_OPCODE_FOR_NAME.values()) + 1
    assert row < 0x20, "custom-DVE row field is 5 bits"
    OPS.append(op)
    CUSTOM_DVE_SPECS[op.name] = op.spec
    _SUB_OPCODE_FOR_NAME[op.name] = row
    for ver in ("v3", "v4"):
        s = DveOpSpec(name=op.name, opcode=row,
                      uops=lower(op.spec, ver=ver), rd1_en=True)
        op.uops_sha[ver] = s.sha(ver)


_register(ADD_WRAP_V)
_register(ADD2_WRAP_V)


# ---- device program -------------------------------------------------------

def _build_program(bc=BC, num_devices=N_CORES, hoist=True):
    ch = min(CH, bc)
    nch = bc // ch             # input/output chunks
    tpc = ch // BT             # tiles per chunk
    nc = bacc.Bacc(
        "TRN2",
        target_bir_lowering=False,
        debug=False,
        num_devices=num_devices,
    )
    v0_d = nc.dram_tensor("v0", [P, 2, bc], F32, kind="ExternalInput").ap()
    s0_d = nc.dram_tensor("s0", [P, 2, bc], BF16, kind="ExternalInput").ap()
    w1_d = nc.dram_tensor("w1", [D1E, D1], BF16, kind="ExternalInput").ap()
    w2_d = nc.dram_tensor("w2", [D1E, D1], BF16, kind="ExternalInput").ap()
    w6_d = nc.dram_tensor("w6", [D1E, D1], BF16, kind="ExternalInput").ap()
    v1_d = nc.dram_tensor("v1", [D2E, D2E], BF16, kind="ExternalInput").ap()
    v2_d = nc.dram_tensor("v2", [D2E, D2E], BF16, kind="ExternalInput").ap()
    v6_d = nc.dram_tensor("v6", [D2E, D2E], BF16, kind="ExternalInput").ap()
    # output rows = stage-2 b-bank partitions 64:80 (class nodes at 69:79)
    out_d = nc.dram_tensor("out", [16, bc], F32, kind="ExternalOutput").ap()

    with tile.TileContext(nc) as tc:
        with (
            tc.tile_pool(name="wts", bufs=1) as wp,
            tc.tile_pool(name="io", bufs=3) as io,
            tc.tile_pool(name="sq", bufs=5) as sq,
            tc.tile_pool(name="psAB", bufs=2, space=bass.MemorySpace.PSUM) as psAB,
            tc.tile_pool(name="psCD", bufs=2, space=bass.MemorySpace.PSUM) as psCD,
        ):
            def load_w(dram, rows, name):
                ta = wp.tile([P, dram.shape[1]], BF16, tag=name + "a")
                nc.sync.dma_start(ta[:], dram[0:P, :])
                tb = wp.tile([rows - P, dram.shape[1]], BF16, tag=name + "b")
                nc.sync.dma_start(tb[:], dram[P:rows, :])
                return ta, tb

            def load_chunk(c, mid=None):
                """Input chunk c: wide-packed [128, 2, ch] (slot 0 = state
                rows 0:128, slot 1 = rows 128:196 + ones row + zero pad).
                `mid` runs between the two DMAs (chunk 0 slots the w1
                load there: the first matmul needs s0+w1, the first wrap
                needs v0 only later)."""
                ccs = slice(c * ch, (c + 1) * ch)
                sm = io.tile([P, 2, ch], BF16, tag="sm")
                nc.sync.dma_start(sm[:], s0_d[:, :, ccs])
                r = mid() if mid else None
                vm = io.tile([P, 2, ch], F32, tag="vm")
                nc.sync.dma_start(vm[:], v0_d[:, :, ccs])
                return (vm, sm) if mid is None else (vm, sm, r)

            vm0, sm0, (w1a, w1b) = load_chunk(
                0, mid=lambda: load_w(w1_d, D1E, "w1"))
            nxt = (vm0, sm0)
            v1a, v1b = load_w(v1_d, D2E, "v1")
            w2a, w2b = load_w(w2_d, D1E, "w2")
            w6a, w6b = load_w(w6_d, D1E, "w6")
            v2a, v2b = load_w(v2_d, D2E, "v2")
            v6a, v6b = load_w(v6_d, D2E, "v6")

            def wsin(src, name):
                """One wide sin over a [128, 2, BT] wrapped-state tile.
                Constant rows self-maintain: the 0.5 driver rows become
                sin(pi/2) = 1 (the folded-bias ones rows), zero pad rows
                become 0."""
                t = sq.tile([P, 2, BT], BF16, tag=name)
                nc.scalar.activation(t[:], src[:], AF.Sin, scale=PI_SIN)
                return t

            def wwrap(op, bank, base, name):
                """One wide [128, 2, BT] custom-DVE op over a merged
                2-bank PSUM tile; slot-1 rows past the live region are
                zeros (one-time bank init + host zero padding)."""
                w = sq.tile([P, 2, BT], F32, tag="wr" + name)
                nc.vector._custom_dve(op, out=w[:], in0=bank[:],
                                      in1=base, s0=0.0, s1=0.0)
                return w

            def emit_st1(t01, chunk, fresh, fresh_ps):
                vm, sm = chunk
                cs = slice((t01 % tpc) * BT, (t01 % tpc + 1) * BT)
                xm = vm[:, :, cs]
                s0a, s0b = sm[:, 0, cs], sm[0:D1KB, 1, cs]
                pab = psAB.tile([P, 2, BT], F32, tag="AB")
                if fresh_ps:
                    # matmuls never touch slot-1 rows 68:128; zero them so
                    # the wide wraps see finite values (and the class rows
                    # of z2 come out exactly zero)
                    nc.scalar.memzero(pab[64:P, 1, :])
                pa, pb = pab[:, 0, :], pab[0:D1B, 1, :]

                def s1_pass(wta, wtb, ra, rb, start=False, stop=False):
                    nc.tensor.matmul(pa, wta[:, 0:P], ra,
                                     start=start, stop=stop,
                                     skip_group_check=True)
                    nc.tensor.matmul(pa, wtb[:, 0:P], rb,
                                     start=False, stop=stop,
                                     skip_group_check=True)
                    nc.tensor.matmul(pb, wta[:, P:D1], ra,
                                     start=start, stop=stop,
                                     skip_group_check=True)
                    nc.tensor.matmul(pb, wtb[:, P:D1], rb,
                                     start=False, stop=stop,
                                     skip_group_check=True)

                s1_pass(w1a, w1b, s0a, s0b, start=True)      # g0 + e
                u2 = wwrap(ADD_WRAP_V, pab, xm, "2")
                s2 = wsin(u2, "s2")
                s2a, s2b = s2[:, 0, :], s2[0:D1KB, 1, :]
                u3 = wwrap(ADD2_WRAP_V, pab, u2[:], "3")
                s3 = wsin(u3, "s3")
                s3a, s3b = s3[:, 0, :], s3[0:D1KB, 1, :]
                s1_pass(w6a, w6b, s0a, s0b)                  # -> 7g0, 7e
                s1_pass(w2a, w2b, s2a, s2b)                  # + 2g2, 9e
                s1_pass(w1a, w1b, s3a, s3b, stop=True)       # + g3, 10e
                # z2 = wrap(v0 + bank); slot-1 rows 68:78 are exactly the
                # class zeros (0 + 0)
                return wwrap(ADD_WRAP_V, pab, xm, "z")

            def emit_st2(t01, z2, outst, fresh, fresh_ps):
                cs = slice((t01 % tpc) * BT, (t01 % tpc + 1) * BT)
                t0 = wsin(z2, "t0")
                t0a, t0b = t0[:, 0, :], t0[0:D2KB, 1, :]
                yab = psCD.tile([P, 2, BT], F32, tag="CD")
                if fresh_ps:
                    nc.scalar.memzero(yab[64:P, 1, :])
                ya, yb = yab[:, 0, :], yab[0:D2KB, 1, :]

                def s2_trim(wta, wtb, ra, rb, stop=False):
                    nc.tensor.matmul(yb, wta[:, P:D2E], ra,
                                     start=False, stop=stop,
                                     skip_group_check=True)
                    nc.tensor.matmul(yb, wtb[:, P:D2E], rb,
                                     start=False, stop=stop,
                                     skip_group_check=True)

                # g0' + e': ya's group opens and closes here (it is only
                # read afterwards); yb keeps accumulating
                nc.tensor.matmul(ya, v1a[:, 0:P], t0a,
                                 start=True, stop=False,
                                 skip_group_check=True)
                nc.tensor.matmul(ya, v1b[:, 0:P], t0b,
                                 start=False, stop=True,
                                 skip_group_check=True)
                nc.tensor.matmul(yb, v1a[:, P:D2E], t0a,
                                 start=True, stop=False,
                                 skip_group_check=True)
                nc.tensor.matmul(yb, v1b[:, P:D2E], t0b,
                                 start=False, stop=False,
                                 skip_group_check=True)
                n2 = wwrap(ADD_WRAP_V, yab, z2[:], "2p")
                t2 = wsin(n2, "t2")
                t2a, t2b = t2[:, 0, :], t2[0:D2KB, 1, :]
                n3 = wwrap(ADD2_WRAP_V, yab, n2[:], "3p")
                t3 = wsin(n3, "t3")
                t3a, t3b = t3[:, 0, :], t3[0:D2KB, 1, :]
                s2_trim(v6a, v6b, t0a, t0b)                  # -> 7g0'
                s2_trim(v2a, v2b, t2a, t2b)                  # + 2g2'
                s2_trim(v1a, v1b, t3a, t3b, stop=True)       # + g3'
                nc.scalar.copy(outst[64:80, cs], yab[64:80, 1, :])
                nc.sync.dma_start(
                    out_d[:, (t01 // tpc) * ch + (t01 % tpc) * BT:
                          (t01 // tpc) * ch + (t01 % tpc + 1) * BT],
                    outst[64:80, cs])

            # software-pipelined emission: stage-1 of tile t runs ahead of
            # stage-2 of tile t-1 so each engine's in-order queue interleaves
            # the two stages instead of head-of-line blocking on tile t-1's
            # tail ops
            ntiles = nch * tpc
            chunks = {0: nxt}
            outsts = {}
            pend = {}
            for t in range(ntiles + 2):
                if t >= 2:
                    tp = t - 2
                    cp = tp // tpc
                    if tp % tpc == 0:
                        outsts[cp] = io.tile([80, ch], F32, tag="outst",
                                             name=f"outst{cp}")
                if t < ntiles:
                    c = t // tpc
                    if t % tpc == 0 and c + 1 < nch:
                        chunks[c + 1] = load_chunk(c + 1)
                    fresh1 = (not hoist) or t < 4
                    fps1 = (not hoist) or t < 2
                    pend[t] = emit_st1(t, chunks[c], fresh1, fps1)
                if t >= 2:
                    tp = t - 2
                    cp = tp // tpc
                    fresh2 = (not hoist) or tp < 4
                    fps2 = (not hoist) or tp < 2
                    emit_st2(tp, pend.pop(tp), outsts[cp], fresh2, fps2)
                    if tp % tpc == tpc - 1:
                        outsts.pop(cp)

    nc.compile()
    return nc


# ---- host side ------------------------------------------------------------

def _c2q(C):
    Q = 0.5 * (C + C.T)
    d = -Q.sum(axis=0)
    Q = Q.copy()
    Q[np.diag_indices_from(Q)] = d
    return Q


def _permute_v(v):
    """Stage-2 weight layout for the wide-packed state: contraction rows
    [state 0:196 | e | class 196:206] (the e-row rides the 0.5-driver at
    slot-1 row 68), target cols [0:196 | dummy zero | class] so the
    class targets land at slot-1 partitions 69:79."""
    rows = np.concatenate([v[0:D2 - NOUT], v[D2:D2 + 1],
                           v[D2 - NOUT:D2]], axis=0)
    return np.insert(rows, D2 - NOUT, 0.0, axis=1)


def _host_weights(fc_w, fc_b, qn, dim):
    """v-units dynamics matrix (scaled by 1.1*DT^2/pi), bias as last row."""
    W = (SC * DT2 / PI) * (_c2q(np.asarray(fc_w, np.float64))
                           + np.asarray(qn, np.float64) - np.eye(dim))
    eb = (SC * DT2 / PI) * np.asarray(fc_b, np.float64)
    return np.concatenate([W, eb[None, :]], axis=0)


def kernel(x, fc1_w, fc1_b, fc2_w, fc2_b, output_fac,
           Q_noise_small, Q_noise_large):
    global LAST_RESULTS
    if "nc" not in _CACHE:
        _CACHE["nc"] = _build_program()
    nc = _CACHE["nc"]

    w1 = _host_weights(fc1_w, fc1_b, Q_noise_small, D1)
    v1 = _host_weights(fc2_w, fc2_b, Q_noise_large, D2)
    v1 = _permute_v(v1)

    BF = ml_dtypes.bfloat16

    def bf(a):
        return np.ascontiguousarray(np.asarray(a, np.float32).astype(BF))

    # v0 = 1.1*x/pi wrapped into [-1, 1]; s0 = sin(1.1 x) exactly.  Both
    # ship wide-packed [128, 2, B]: slot 0 = state rows 0:128, slot 1 =
    # rows 128:196 (+ a ones row in s0 for the folded e-bias, zero pad)
    u = (SC / PI) * np.asarray(x, np.float64)
    u = u - 2.0 * ((u > 1.0).astype(np.float64) - (u < -1.0).astype(np.float64))
    vt = np.asarray(u.T, np.float32)                   # [D1, B]
    st = np.sin(PI * np.asarray(u.T, np.float64)).astype(np.float32)
    nb = vt.shape[1]
    vm = np.zeros((P, 2, nb), np.float32)
    vm[:, 0, :] = vt[0:P]
    vm[0:D1B, 1, :] = vt[P:D1]
    vm[D1B, 1, :] = 0.5          # ones-driver: every wrap keeps it 0.5,
    sm = np.zeros((P, 2, nb), np.float32)   # every sin turns it into 1.0
    sm[:, 0, :] = st[0:P]
    sm[0:D1B, 1, :] = st[P:D1]
    sm[D1B, 1, :] = 1.0

    common = {
        "w1": bf(w1), "w2": bf(2.0 * w1), "w6": bf(6.0 * w1),
        "v1": bf(v1), "v2": bf(2.0 * v1), "v6": bf(6.0 * v1),
    }
    in_maps = []
    for c in range(N_CORES):
        m = dict(common)
        m["v0"] = np.ascontiguousarray(vm[:, :, c * BC:(c + 1) * BC])
        m["s0"] = np.ascontiguousarray(sm[:, :, c * BC:(c + 1) * BC]).astype(BF)
        in_maps.append(m)

    res = None
    last_exc = None
    for _attempt in range(3):
        try:
            res = run_bass_kernel_spmd(
                nc, in_maps, core_ids=list(range(N_CORES)), trace=TRACE)
            break
        except Exception as e:  # transient NRT/device hiccups
            last_exc = e
            try:
                import time as _time

                import jax as _jax
                _jax.clear_caches()
                if hasattr(_jax, "clear_backends"):
                    _jax.clear_backends()
                _time.sleep(5)
            except Exception:
                pass
    if res is None:
        raise last_exc
    LAST_RESULTS = res

    out = np.empty((B, NOUT), np.float32)
    for c in range(N_CORES):
        out[c * BC:(c + 1) * BC, :] = res.results[c]["out"][5:15, :].T
    fac = float(np.asarray(output_fac)) * PI / SC
    return out * np.float32(fac)


# revision 48
# speedup vs baseline: 1.0044x; 1.0003x over previous
"""Trainium2 Bass kernel for nn_Net_75282186764473.

Math: reference pat() returns zm + stop_gradient(ze - zm) == ze numerically;
the forward pass is 5 explicit-Euler steps of the 'experiment' dynamics per
stage, twice:  q' = p ; p' = sin(1.1 q) @ (c2q(C) + Qn - I) + e.
With u = 1.1 q, g_n = sin(u_n) @ W + eb  (W, eb scaled by 1.1*DT^2):
    u2 = u0 + g0 ; u3 = u0 + 3 g0 ; u5 = u0 + 7 g0 + 2 g2 + g3
so each stage needs sins at u0, u2, u3 and weighted passes {1,6,2,1}*W:
the 6W pass tops the g0 PSUM bank up to 7g0 after its last read, then the
2W/1W passes accumulate 2g2 + g3 into the same bank.

All state lives in v = u/pi units so the range wrap is 'bound +-1,
period 2', which two custom DVE ops implement with zero constant slots:
    ADD_WRAP_V  : out = wrap(in0 + in1)
    ADD2_WRAP_V : out = wrap(2*in0 + in1)
(u3 = wrap(u2_wrapped + 2*g0) is exact mod 2.)  sin(u) = Act Sin with
scale=pi on the v-state, so the table sin stays exact; weights carry the
1/pi.  e-biases ride as an extra contraction row driven by a constant
0.5 in the wide state (every wrap maps 0.5 -> 0.5, every sin -> 1.0).

Layout: everything is wide-packed [128, 2, 512] per batch tile -- slot 0
= node rows 0:128, slot 1 = rows 128:196 + driver/class/pad rows -- so
each wrap is ONE wide DVE op over a merged 2-bank PSUM tile and each sin
is ONE wide Act op.  Stage 2 permutes the weights (_permute_v) so its
slot-1 contraction is [state 68 | e-row | class 10] and its targets are
[state 68 | dummy | class 10]: the class-node zeros fall out of zeroed
bank rows + zero input padding automatically.  Per tile: 26 matmuls
(512-col, bf16), 5 wide wraps (DVE), 5 wide sins + 1 copy (Act).
sin(1.1 x) is precomputed on the host and shipped as bf16 next to the
fp32 v0 state.  Stage-1 emission runs two tiles ahead of stage-2
(software pipelining) so the in-order engine queues stay fed.

Sharding: pure batch data-parallel across 8 cores (8192 rows each); the
outputs are PSUM slot-1 partitions 69:79 of the stage-2 bank, scaled by
pi/1.1 on the host.
"""

import ml_dtypes
import numpy as np

import concourse.bacc as bacc
import concourse.bass as bass
import concourse.mybir as mybir
import concourse.tile as tile
from concourse.bass_utils import run_bass_kernel_spmd
from concourse.dve_ops import (
    CUSTOM_DVE_SPECS,
    OPS,
    DveOp,
    _SUB_OPCODE_FOR_NAME,
)
from concourse.dve_spec import One, Spec, Src0, Src1, Zero, lower
from concourse.dve_uop import DveOpSpec

AF = mybir.ActivationFunctionType
F32 = mybir.dt.float32
BF16 = mybir.dt.bfloat16

N_CORES = 8
B = 65536
BC = B // N_CORES          # 8192 batch rows per core
D1 = 196                   # stage-1 nodes
D1E = 197                  # + bias row
D2 = 206                   # stage-2 nodes (+10 class)
D2E = 207
P = 128
D1B = D1 - P               # 68
D1KB = D1E - P             # 69
D2B = D2 - P               # 78
D2KB = D2E - P             # 79
NOUT = 10
BT = 512                   # batch tile (one PSUM bank of fp32)
CH = 1024                  # input/output DMA chunk (2 tiles)
SC = 1.1                   # sin argument scale (1 + eta)
DT = 0.5 / 5
DT2 = DT * DT
PI = float(np.pi)
# sin scale: a hair under pi so wrapped values at exactly +-1 stay inside
# the Act table's [-pi, pi] domain after the fp32 multiply
PI_SIN = float(np.float32(np.pi) * (1.0 - 3e-7))

TRACE = False
LAST_RESULTS = None

_CACHE = {}


# ---- custom DVE ops: +-1 bound / period-2 wrap in v = u/pi units ----------

def _wrap1(y):
    d = (y < (Zero - One)) - (One < y)
    return (y + d) + d


def _np_wrap(y):
    y = np.asarray(y, np.float32)
    return y + 2.0 * ((y < -1.0).astype(np.float32)
                      - (y > 1.0).astype(np.float32))


def _flat2(a, b):
    """CoreSim may pass the two operands with different (coalesced vs
    multi-dim) shapes; compare them [P, -1]."""
    a = np.asarray(a, np.float32)
    b = np.asarray(b, np.float32)
    return a.reshape(a.shape[0], -1), b.reshape(a.shape[0], -1)


ADD_WRAP_V = DveOp(
    "ADD_WRAP_V",
    Spec(
        body=_wrap1(Src0 + Src1),
        reference=lambda in0, in1, s0, s1, imm2: _np_wrap(
            sum(_flat2(in0, in1))),
    ),
    subdim=False,
    uops_sha={},
)

ADD2_WRAP_V = DveOp(
    "ADD2_WRAP_V",
    Spec(
        body=_wrap1((Src0 + Src0) + Src1),
        reference=lambda in0, in1, s0, s1, imm2: _np_wrap(
            (lambda a, b: 2.0 * a + b)(*_flat2(in0, in1))),
    ),
    subdim=False,
    uops_sha={},
)



def _register(op):
    if op.name in _SUB_OPCODE_FOR_NAME:
        return
    row = max(_SUB_OPCODE_FOR_NAME.values()) + 1
    assert row < 0x20, "custom-DVE row field is 5 bits"
    OPS.append(op)
    CUSTOM_DVE_SPECS[op.name] = op.spec
    _SUB_OPCODE_FOR_NAME[op.name] = row
    for ver in ("v3", "v4"):
        s = DveOpSpec(name=op.name, opcode=row,
                      uops=lower(op.spec, ver=ver), rd1_en=True)
        op.uops_sha[ver] = s.sha(ver)


_register(ADD_WRAP_V)
_register(ADD2_WRAP_V)


# ---- device program -------------------------------------------------------

def _build_program(bc=BC, num_devices=N_CORES, hoist=True):
    ch = min(CH, bc)
    nch = bc // ch             # input/output chunks
    tpc = ch // BT             # tiles per chunk
    nc = bacc.Bacc(
        "TRN2",
        target_bir_lowering=False,
        debug=False,
        num_devices=num_devices,
    )
    v0_d = nc.dram_tensor("v0", [P, 2, bc], F32, kind="ExternalInput").ap()
    s0_d = nc.dram_tensor("s0", [P, 2, bc], BF16, kind="ExternalInput").ap()
    w1_d = nc.dram_tensor("w1", [D1E, D1], BF16, kind="ExternalInput").ap()
    w2_d = nc.dram_tensor("w2", [D1E, D1], BF16, kind="ExternalInput").ap()
    w6_d = nc.dram_tensor("w6", [D1E, D1], BF16, kind="ExternalInput").ap()
    v1_d = nc.dram_tensor("v1", [D2E, D2E], BF16, kind="ExternalInput").ap()
    v2_d = nc.dram_tensor("v2", [D2E, D2E], BF16, kind="ExternalInput").ap()
    v6_d = nc.dram_tensor("v6", [D2E, D2E], BF16, kind="ExternalInput").ap()
    # output rows = stage-2 b-bank partitions 64:80 (class nodes at 69:79)
    out_d = nc.dram_tensor("out", [16, bc], F32, kind="ExternalOutput").ap()

    with tile.TileContext(nc) as tc:
        with (
            tc.tile_pool(name="wts", bufs=1) as wp,
            tc.tile_pool(name="io", bufs=3) as io,
            tc.tile_pool(name="sq", bufs=5) as sq,
            tc.tile_pool(name="psAB", bufs=2, space=bass.MemorySpace.PSUM) as psAB,
            tc.tile_pool(name="psCD", bufs=2, space=bass.MemorySpace.PSUM) as psCD,
        ):
            def load_w(dram, rows, name):
                ta = wp.tile([P, dram.shape[1]], BF16, tag=name + "a")
                nc.sync.dma_start(ta[:], dram[0:P, :])
                tb = wp.tile([rows - P, dram.shape[1]], BF16, tag=name + "b")
                nc.sync.dma_start(tb[:], dram[P:rows, :])
                return ta, tb

            def load_chunk(c, mid=None):
                """Input chunk c: wide-packed [128, 2, ch] (slot 0 = state
                rows 0:128, slot 1 = rows 128:196 + ones row + zero pad).
                `mid` runs between the two DMAs (chunk 0 slots the w1
                load there: the first matmul needs s0+w1, the first wrap
                needs v0 only later)."""
                ccs = slice(c * ch, (c + 1) * ch)
                sm = io.tile([P, 2, ch], BF16, tag="sm")
                nc.sync.dma_start(sm[:], s0_d[:, :, ccs])
                r = mid() if mid else None
                vm = io.tile([P, 2, ch], F32, tag="vm")
                nc.sync.dma_start(vm[:], v0_d[:, :, ccs])
                return (vm, sm) if mid is None else (vm, sm, r)

            vm0, sm0, (w1a, w1b) = load_chunk(
                0, mid=lambda: load_w(w1_d, D1E, "w1"))
            nxt = (vm0, sm0)
            v1a, v1b = load_w(v1_d, D2E, "v1")
            # PE p-state warm-up: dummy matmuls into the first pab slot
            # while the input DMAs are in flight, so the first real pass
            # runs at full clock (the bank is reset by its start=True)
            warm = wp.tile([1, BT], BF16, tag="warm")
            nc.gpsimd.memset(warm[:], 0.0)
            pwu = psAB.tile([P, 2, BT], F32, tag="AB", name="pwu")
            for _ in range(14):
                nc.tensor.matmul(pwu[:, 0, :], warm[0:1, 0:P], warm[:],
                                 start=True, stop=True,
                                 skip_group_check=True)
            w2a, w2b = load_w(w2_d, D1E, "w2")
            w6a, w6b = load_w(w6_d, D1E, "w6")
            v2a, v2b = load_w(v2_d, D2E, "v2")
            v6a, v6b = load_w(v6_d, D2E, "v6")

            def wsin(src, name):
                """One wide sin over a [128, 2, BT] wrapped-state tile.
                Constant rows self-maintain: the 0.5 driver rows become
                sin(pi/2) = 1 (the folded-bias ones rows), zero pad rows
                become 0."""
                t = sq.tile([P, 2, BT], BF16, tag=name)
                nc.scalar.activation(t[:], src[:], AF.Sin, scale=PI_SIN)
                return t

            def wwrap(op, bank, base, name):
                """One wide [128, 2, BT] custom-DVE op over a merged
                2-bank PSUM tile; slot-1 rows past the live region are
                zeros (one-time bank init + host zero padding)."""
                w = sq.tile([P, 2, BT], F32, tag="wr" + name)
                nc.vector._custom_dve(op, out=w[:], in0=bank[:],
                                      in1=base, s0=0.0, s1=0.0)
                return w

            def emit_st1(t01, chunk, fresh, fresh_ps):
                vm, sm = chunk
                cs = slice((t01 % tpc) * BT, (t01 % tpc + 1) * BT)
                xm = vm[:, :, cs]
                s0a, s0b = sm[:, 0, cs], sm[0:D1KB, 1, cs]
                pab = psAB.tile([P, 2, BT], F32, tag="AB")
                if fresh_ps:
                    # matmuls never touch slot-1 rows 68:128; zero them so
                    # the wide wraps see finite values (and the class rows
                    # of z2 come out exactly zero)
                    nc.scalar.memzero(pab[64:P, 1, :])
                pa, pb = pab[:, 0, :], pab[0:D1B, 1, :]

                def s1_pass(wta, wtb, ra, rb, start=False, stop=False):
                    nc.tensor.matmul(pa, wta[:, 0:P], ra,
                                     start=start, stop=stop,
                                     skip_group_check=True)
                    nc.tensor.matmul(pa, wtb[:, 0:P], rb,
                                     start=False, stop=stop,
                                     skip_group_check=True)
                    nc.tensor.matmul(pb, wta[:, P:D1], ra,
                                     start=start, stop=stop,
                                     skip_group_check=True)
                    nc.tensor.matmul(pb, wtb[:, P:D1], rb,
                                     start=False, stop=stop,
                                     skip_group_check=True)

                s1_pass(w1a, w1b, s0a, s0b, start=True)      # g0 + e
                u2 = wwrap(ADD_WRAP_V, pab, xm, "2")
                s2 = wsin(u2, "s2")
                s2a, s2b = s2[:, 0, :], s2[0:D1KB, 1, :]
                u3 = wwrap(ADD2_WRAP_V, pab, u2[:], "3")
                s3 = wsin(u3, "s3")
                s3a, s3b = s3[:, 0, :], s3[0:D1KB, 1, :]
                s1_pass(w6a, w6b, s0a, s0b)                  # -> 7g0, 7e
                s1_pass(w2a, w2b, s2a, s2b)                  # + 2g2, 9e
                s1_pass(w1a, w1b, s3a, s3b, stop=True)       # + g3, 10e
                # z2 = wrap(v0 + bank); slot-1 rows 68:78 are exactly the
                # class zeros (0 + 0)
                return wwrap(ADD_WRAP_V, pab, xm, "z")

            def emit_st2(t01, z2, outst, fresh, fresh_ps):
                cs = slice((t01 % tpc) * BT, (t01 % tpc + 1) * BT)
                t0 = wsin(z2, "t0")
                t0a, t0b = t0[:, 0, :], t0[0:D2KB, 1, :]
                yab = psCD.tile([P, 2, BT], F32, tag="CD")
                if fresh_ps:
                    nc.scalar.memzero(yab[64:P, 1, :])
                ya, yb = yab[:, 0, :], yab[0:D2KB, 1, :]

                def s2_trim(wta, wtb, ra, rb, stop=False):
                    nc.tensor.matmul(yb, wta[:, P:D2E], ra,
                                     start=False, stop=stop,
                                     skip_group_check=True)
                    nc.tensor.matmul(yb, wtb[:, P:D2E], rb,
                                     start=False, stop=stop,
                                     skip_group_check=True)

                # g0' + e': ya's group opens and closes here (it is only
                # read afterwards); yb keeps accumulating
                nc.tensor.matmul(ya, v1a[:, 0:P], t0a,
                                 start=True, stop=False,
                                 skip_group_check=True)
                nc.tensor.matmul(ya, v1b[:, 0:P], t0b,
                                 start=False, stop=True,
                                 skip_group_check=True)
                nc.tensor.matmul(yb, v1a[:, P:D2E], t0a,
                                 start=True, stop=False,
                                 skip_group_check=True)
                nc.tensor.matmul(yb, v1b[:, P:D2E], t0b,
                                 start=False, stop=False,
                                 skip_group_check=True)
                n2 = wwrap(ADD_WRAP_V, yab, z2[:], "2p")
                t2 = wsin(n2, "t2")
                t2a, t2b = t2[:, 0, :], t2[0:D2KB, 1, :]
                n3 = wwrap(ADD2_WRAP_V, yab, n2[:], "3p")
                t3 = wsin(n3, "t3")
                t3a, t3b = t3[:, 0, :], t3[0:D2KB, 1, :]
                s2_trim(v6a, v6b, t0a, t0b)                  # -> 7g0'
                s2_trim(v2a, v2b, t2a, t2b)                  # + 2g2'
                s2_trim(v1a, v1b, t3a, t3b, stop=True)       # + g3'
                nc.scalar.copy(outst[64:80, cs], yab[64:80, 1, :])
                nc.sync.dma_start(
                    out_d[:, (t01 // tpc) * ch + (t01 % tpc) * BT:
                          (t01 // tpc) * ch + (t01 % tpc + 1) * BT],
                    outst[64:80, cs])

            # software-pipelined emission: stage-1 of tile t runs ahead of
            # stage-2 of tile t-1 so each engine's in-order queue interleaves
            # the two stages instead of head-of-line blocking on tile t-1's
            # tail ops
            ntiles = nch * tpc
            chunks = {0: nxt}
            outsts = {}
            pend = {}
            for t in range(ntiles + 2):
                if t >= 2:
                    tp = t - 2
                    cp = tp // tpc
                    if tp % tpc == 0:
                        outsts[cp] = io.tile([80, ch], F32, tag="outst",
                                             name=f"outst{cp}")
                if t < ntiles:
                    c = t // tpc
                    if t % tpc == 0 and c + 1 < nch:
                        chunks[c + 1] = load_chunk(c + 1)
                    fresh1 = (not hoist) or t < 4
                    fps1 = (not hoist) or t < 2
                    pend[t] = emit_st1(t, chunks[c], fresh1, fps1)
                if t >= 2:
                    tp = t - 2
                    cp = tp // tpc
                    fresh2 = (not hoist) or tp < 4
                    fps2 = (not hoist) or tp < 2
                    emit_st2(tp, pend.pop(tp), outsts[cp], fresh2, fps2)
                    if tp % tpc == tpc - 1:
                        outsts.pop(cp)

    nc.compile()
    return nc


# ---- host side ------------------------------------------------------------

def _c2q(C):
    Q = 0.5 * (C + C.T)
    d = -Q.sum(axis=0)
    Q = Q.copy()
    Q[np.diag_indices_from(Q)] = d
    return Q


def _permute_v(v):
    """Stage-2 weight layout for the wide-packed state: contraction rows
    [state 0:196 | e | class 196:206] (the e-row rides the 0.5-driver at
    slot-1 row 68), target cols [0:196 | dummy zero | class] so the
    class targets land at slot-1 partitions 69:79."""
    rows = np.concatenate([v[0:D2 - NOUT], v[D2:D2 + 1],
                           v[D2 - NOUT:D2]], axis=0)
    return np.insert(rows, D2 - NOUT, 0.0, axis=1)


def _host_weights(fc_w, fc_b, qn, dim):
    """v-units dynamics matrix (scaled by 1.1*DT^2/pi), bias as last row."""
    W = (SC * DT2 / PI) * (_c2q(np.asarray(fc_w, np.float64))
                           + np.asarray(qn, np.float64) - np.eye(dim))
    eb = (SC * DT2 / PI) * np.asarray(fc_b, np.float64)
    return np.concatenate([W, eb[None, :]], axis=0)


def kernel(x, fc1_w, fc1_b, fc2_w, fc2_b, output_fac,
           Q_noise_small, Q_noise_large):
    global LAST_RESULTS
    if "nc" not in _CACHE:
        _CACHE["nc"] = _build_program()
    nc = _CACHE["nc"]

    w1 = _host_weights(fc1_w, fc1_b, Q_noise_small, D1)
    v1 = _host_weights(fc2_w, fc2_b, Q_noise_large, D2)
    v1 = _permute_v(v1)

    BF = ml_dtypes.bfloat16

    def bf(a):
        return np.ascontiguousarray(np.asarray(a, np.float32).astype(BF))

    # v0 = 1.1*x/pi wrapped into [-1, 1]; s0 = sin(1.1 x) exactly.  Both
    # ship wide-packed [128, 2, B]: slot 0 = state rows 0:128, slot 1 =
    # rows 128:196 (+ a ones row in s0 for the folded e-bias, zero pad)
    u = (SC / PI) * np.asarray(x, np.float64)
    u = u - 2.0 * ((u > 1.0).astype(np.float64) - (u < -1.0).astype(np.float64))
    vt = np.asarray(u.T, np.float32)                   # [D1, B]
    st = np.sin(PI * np.asarray(u.T, np.float64)).astype(np.float32)
    nb = vt.shape[1]
    vm = np.zeros((P, 2, nb), np.float32)
    vm[:, 0, :] = vt[0:P]
    vm[0:D1B, 1, :] = vt[P:D1]
    vm[D1B, 1, :] = 0.5          # ones-driver: every wrap keeps it 0.5,
    sm = np.zeros((P, 2, nb), np.float32)   # every sin turns it into 1.0
    sm[:, 0, :] = st[0:P]
    sm[0:D1B, 1, :] = st[P:D1]
    sm[D1B, 1, :] = 1.0

    common = {
        "w1": bf(w1), "w2": bf(2.0 * w1), "w6": bf(6.0 * w1),
        "v1": bf(v1), "v2": bf(2.0 * v1), "v6": bf(6.0 * v1),
    }
    in_maps = []
    for c in range(N_CORES):
        m = dict(common)
        m["v0"] = np.ascontiguousarray(vm[:, :, c * BC:(c + 1) * BC])
        m["s0"] = np.ascontiguousarray(sm[:, :, c * BC:(c + 1) * BC]).astype(BF)
        in_maps.append(m)

    res = None
    last_exc = None
    for _attempt in range(3):
        try:
            res = run_bass_kernel_spmd(
                nc, in_maps, core_ids=list(range(N_CORES)), trace=TRACE)
            break
        except Exception as e:  # transient NRT/device hiccups
            last_exc = e
            try:
                import time as _time

                import jax as _jax
                _jax.clear_caches()
                if hasattr(_jax, "clear_backends"):
                    _jax.clear_backends()
                _time.sleep(5)
            except Exception:
                pass
    if res is None:
        raise last_exc
    LAST_RESULTS = res

    out = np.empty((B, NOUT), np.float32)
    for c in range(N_CORES):
        out[c * BC:(c + 1) * BC, :] = res.results[c]["out"][5:15, :].T
    fac = float(np.asarray(output_fac)) * PI / SC
    return out * np.float32(fac)


# revision 53
# speedup vs baseline: 1.0322x; 1.0277x over previous
"""Trainium2 Bass kernel for nn_Net_75282186764473.

Math: reference pat() returns zm + stop_gradient(ze - zm) == ze numerically;
the forward pass is 5 explicit-Euler steps of the 'experiment' dynamics per
stage, twice:  q' = p ; p' = sin(1.1 q) @ (c2q(C) + Qn - I) + e.
With u = 1.1 q, g_n = sin(u_n) @ W + eb  (W, eb scaled by 1.1*DT^2):
    u2 = u0 + g0 ; u3 = u0 + 3 g0 ; u5 = u0 + 7 g0 + 2 g2 + g3
so each stage needs sins at u0, u2, u3 and weighted passes {1,6,2,1}*W:
the 6W pass tops the g0 PSUM bank up to 7g0 after its last read, then the
2W/1W passes accumulate 2g2 + g3 into the same bank.

All state lives in v = u/pi units so the range wrap is 'bound +-1,
period 2', which two custom DVE ops implement with zero constant slots:
    ADD_WRAP_V  : out = wrap(in0 + in1)
    ADD2_WRAP_V : out = wrap(2*in0 + in1)
(u3 = wrap(u2_wrapped + 2*g0) is exact mod 2.)  sin(u) = Act Sin with
scale=pi on the v-state, so the table sin stays exact; weights carry the
1/pi.  e-biases ride as an extra contraction row driven by a constant
0.5 in the wide state (every wrap maps 0.5 -> 0.5, every sin -> 1.0).

Layout: everything is wide-packed [128, 2, 512] per batch tile -- slot 0
= node rows 0:128, slot 1 = rows 128:196 + driver/class/pad rows -- so
each wrap is ONE wide DVE op over a merged 2-bank PSUM tile and each sin
is ONE wide Act op.  Stage 2 permutes the weights (_permute_v) so its
slot-1 contraction is [state 68 | e-row | class 10] and its targets are
[state 68 | dummy | class 10]: the class-node zeros fall out of zeroed
bank rows + zero input padding automatically.  Per tile: 26 matmuls
(512-col, bf16), 5 wide wraps (DVE), 5 wide sins + 1 copy (Act).
sin(1.1 x) is precomputed on the host and shipped as bf16 next to the
fp32 v0 state.  Stage-1 emission runs two tiles ahead of stage-2
(software pipelining) so the in-order engine queues stay fed.

Sharding: pure batch data-parallel across 8 cores (8192 rows each); the
outputs are PSUM slot-1 partitions 69:79 of the stage-2 bank, scaled by
pi/1.1 on the host.
"""

import ml_dtypes
import numpy as np

import concourse.bacc as bacc
import concourse.bass as bass
import concourse.mybir as mybir
import concourse.tile as tile
from concourse.bass_utils import run_bass_kernel_spmd
from concourse.dve_ops import (
    CUSTOM_DVE_SPECS,
    OPS,
    DveOp,
    _SUB_OPCODE_FOR_NAME,
)
from concourse.dve_spec import One, Spec, Src0, Src1, Zero, lower
from concourse.dve_uop import DveOpSpec

AF = mybir.ActivationFunctionType
F32 = mybir.dt.float32
BF16 = mybir.dt.bfloat16

N_CORES = 8
B = 65536
BC = B // N_CORES          # 8192 batch rows per core
D1 = 196                   # stage-1 nodes
D1E = 197                  # + bias row
D2 = 206                   # stage-2 nodes (+10 class)
D2E = 207
P = 128
D1B = D1 - P               # 68
D1KB = D1E - P             # 69
D2B = D2 - P               # 78
D2KB = D2E - P             # 79
NOUT = 10
BT = 512                   # batch tile (one PSUM bank of fp32)
CH = 1024                  # input/output DMA chunk (2 tiles)
SC = 1.1                   # sin argument scale (1 + eta)
DT = 0.5 / 5
DT2 = DT * DT
PI = float(np.pi)
# sin scale: a hair under pi so wrapped values at exactly +-1 stay inside
# the Act table's [-pi, pi] domain after the fp32 multiply
PI_SIN = float(np.float32(np.pi) * (1.0 - 3e-7))

TRACE = False
LAST_RESULTS = None

_CACHE = {}


# ---- custom DVE ops: +-1 bound / period-2 wrap in v = u/pi units ----------

def _wrap1(y):
    d = (y < (Zero - One)) - (One < y)
    return (y + d) + d


def _np_wrap(y):
    y = np.asarray(y, np.float32)
    return y + 2.0 * ((y < -1.0).astype(np.float32)
                      - (y > 1.0).astype(np.float32))


def _flat2(a, b):
    """CoreSim may pass the two operands with different (coalesced vs
    multi-dim) shapes; compare them [P, -1]."""
    a = np.asarray(a, np.float32)
    b = np.asarray(b, np.float32)
    return a.reshape(a.shape[0], -1), b.reshape(a.shape[0], -1)


ADD_WRAP_V = DveOp(
    "ADD_WRAP_V",
    Spec(
        body=_wrap1(Src0 + Src1),
        reference=lambda in0, in1, s0, s1, imm2: _np_wrap(
            sum(_flat2(in0, in1))),
    ),
    subdim=False,
    uops_sha={},
)

ADD2_WRAP_V = DveOp(
    "ADD2_WRAP_V",
    Spec(
        body=_wrap1((Src0 + Src0) + Src1),
        reference=lambda in0, in1, s0, s1, imm2: _np_wrap(
            (lambda a, b: 2.0 * a + b)(*_flat2(in0, in1))),
    ),
    subdim=False,
    uops_sha={},
)

# out = wrap(s0*in0 + in1): reads the 7x-scaled g-bank at 1/7 (u2) and
# 2/7 (u3), so one 7W matmul pass replaces the former 1W + 6W pair
from concourse.dve_spec import C0 as _C0
ADD_SCALE_WRAP_V = DveOp(
    "ADD_SCALE_WRAP_V",
    Spec(
        body=_wrap1(_C0 * Src0 + Src1),
        reference=lambda in0, in1, s0, s1, imm2: _np_wrap(
            (lambda a, b: np.float32(s0) * a + b)(*_flat2(in0, in1))),
    ),
    subdim=False,
    uops_sha={},
)



def _register(op):
    if op.name in _SUB_OPCODE_FOR_NAME:
        return
    row = max(_SUB_OPCODE_FOR_NAME.values()) + 1
    assert row < 0x20, "custom-DVE row field is 5 bits"
    OPS.append(op)
    CUSTOM_DVE_SPECS[op.name] = op.spec
    _SUB_OPCODE_FOR_NAME[op.name] = row
    for ver in ("v3", "v4"):
        s = DveOpSpec(name=op.name, opcode=row,
                      uops=lower(op.spec, ver=ver), rd1_en=True)
        op.uops_sha[ver] = s.sha(ver)


_register(ADD_WRAP_V)
_register(ADD2_WRAP_V)
_register(ADD_SCALE_WRAP_V)


# ---- device program -------------------------------------------------------

def _build_program(bc=BC, num_devices=N_CORES, hoist=True):
    ch = min(CH, bc)
    nch = bc // ch             # input/output chunks
    tpc = ch // BT             # tiles per chunk
    nc = bacc.Bacc(
        "TRN2",
        target_bir_lowering=False,
        debug=False,
        num_devices=num_devices,
    )
    v0_d = nc.dram_tensor("v0", [P, 2, bc], F32, kind="ExternalInput").ap()
    s0_d = nc.dram_tensor("s0", [P, 2, bc], BF16, kind="ExternalInput").ap()
    w1_d = nc.dram_tensor("w1", [D1E, D1], BF16, kind="ExternalInput").ap()
    w2_d = nc.dram_tensor("w2", [D1E, D1], BF16, kind="ExternalInput").ap()
    w6_d = nc.dram_tensor("w6", [D1E, D1], BF16, kind="ExternalInput").ap()
    v1_d = nc.dram_tensor("v1", [D2E, D2E], BF16, kind="ExternalInput").ap()
    v2_d = nc.dram_tensor("v2", [D2E, D2E], BF16, kind="ExternalInput").ap()
    v6_d = nc.dram_tensor("v6", [D2E, D2E], BF16, kind="ExternalInput").ap()
    # output rows = stage-2 b-bank partitions 64:80 (class nodes at 69:79)
    out_d = nc.dram_tensor("out", [16, bc], F32, kind="ExternalOutput").ap()

    with tile.TileContext(nc) as tc:
        with (
            tc.tile_pool(name="wts", bufs=1) as wp,
            tc.tile_pool(name="io", bufs=3) as io,
            tc.tile_pool(name="sq", bufs=5) as sq,
            tc.tile_pool(name="psAB", bufs=2, space=bass.MemorySpace.PSUM) as psAB,
            tc.tile_pool(name="psCD", bufs=2, space=bass.MemorySpace.PSUM) as psCD,
        ):
            def load_w(dram, rows, name):
                ta = wp.tile([P, dram.shape[1]], BF16, tag=name + "a")
                nc.sync.dma_start(ta[:], dram[0:P, :])
                tb = wp.tile([rows - P, dram.shape[1]], BF16, tag=name + "b")
                nc.sync.dma_start(tb[:], dram[P:rows, :])
                return ta, tb

            def load_chunk(c, mid=None):
                """Input chunk c: wide-packed [128, 2, ch] (slot 0 = state
                rows 0:128, slot 1 = rows 128:196 + ones row + zero pad).
                `mid` runs between the two DMAs (chunk 0 slots the w1
                load there: the first matmul needs s0+w1, the first wrap
                needs v0 only later)."""
                ccs = slice(c * ch, (c + 1) * ch)
                sm = io.tile([P, 2, ch], BF16, tag="sm")
                nc.sync.dma_start(sm[:], s0_d[:, :, ccs])
                r = mid() if mid else None
                vm = io.tile([P, 2, ch], F32, tag="vm")
                nc.sync.dma_start(vm[:], v0_d[:, :, ccs])
                return (vm, sm) if mid is None else (vm, sm, r)

            vm0, sm0, (w6a, w6b) = load_chunk(
                0, mid=lambda: load_w(w6_d, D1E, "w6"))
            nxt = (vm0, sm0)
            v6a, v6b = load_w(v6_d, D2E, "v6")
            # PE p-state warm-up: dummy matmuls into the first pab slot
            # while the input DMAs are in flight, so the first real pass
            # runs at full clock (the bank is reset by its start=True)
            warm = wp.tile([1, BT], BF16, tag="warm")
            nc.gpsimd.memset(warm[:], 0.0)
            pwu = psAB.tile([P, 2, BT], F32, tag="AB", name="pwu")
            for _ in range(14):
                nc.tensor.matmul(pwu[:, 0, :], warm[0:1, 0:P], warm[:],
                                 start=True, stop=True,
                                 skip_group_check=True)
            w2a, w2b = load_w(w2_d, D1E, "w2")
            v2a, v2b = load_w(v2_d, D2E, "v2")
            w1a, w1b = load_w(w1_d, D1E, "w1")
            v1a, v1b = load_w(v1_d, D2E, "v1")

            def wsin(src, name):
                """One wide sin over a [128, 2, BT] wrapped-state tile.
                Constant rows self-maintain: the 0.5 driver rows become
                sin(pi/2) = 1 (the folded-bias ones rows), zero pad rows
                become 0."""
                t = sq.tile([P, 2, BT], BF16, tag=name)
                nc.scalar.activation(t[:], src[:], AF.Sin, scale=PI_SIN)
                return t

            def wwrap(op, bank, base, name, k=0.0):
                """One wide [128, 2, BT] custom-DVE op over a merged
                2-bank PSUM tile; slot-1 rows past the live region are
                zeros (one-time bank init + host zero padding)."""
                w = sq.tile([P, 2, BT], F32, tag="wr" + name)
                nc.vector._custom_dve(op, out=w[:], in0=bank[:],
                                      in1=base, s0=k, s1=0.0)
                return w

            def emit_st1(t01, chunk, fresh, fresh_ps):
                vm, sm = chunk
                cs = slice((t01 % tpc) * BT, (t01 % tpc + 1) * BT)
                xm = vm[:, :, cs]
                s0a, s0b = sm[:, 0, cs], sm[0:D1KB, 1, cs]
                pab = psAB.tile([P, 2, BT], F32, tag="AB")
                if fresh_ps:
                    # matmuls never touch slot-1 rows 68:128; zero them so
                    # the wide wraps see finite values (and the class rows
                    # of z2 come out exactly zero)
                    nc.scalar.memzero(pab[64:P, 1, :])
                pa, pb = pab[:, 0, :], pab[0:D1B, 1, :]

                def s1_pass(wta, wtb, ra, rb, start=False, stop=False):
                    nc.tensor.matmul(pa, wta[:, 0:P], ra,
                                     start=start, stop=stop,
                                     skip_group_check=True)
                    nc.tensor.matmul(pa, wtb[:, 0:P], rb,
                                     start=False, stop=stop,
                                     skip_group_check=True)
                    nc.tensor.matmul(pb, wta[:, P:D1], ra,
                                     start=start, stop=stop,
                                     skip_group_check=True)
                    nc.tensor.matmul(pb, wtb[:, P:D1], rb,
                                     start=False, stop=stop,
                                     skip_group_check=True)

                s1_pass(w6a, w6b, s0a, s0b, start=True)      # 7(g0 + e)
                u2 = wwrap(ADD_SCALE_WRAP_V, pab, xm, "2", k=1.0 / 7.0)
                s2 = wsin(u2, "s2")
                s2a, s2b = s2[:, 0, :], s2[0:D1KB, 1, :]
                u3 = wwrap(ADD_SCALE_WRAP_V, pab, u2[:], "3", k=2.0 / 7.0)
                s3 = wsin(u3, "s3")
                s3a, s3b = s3[:, 0, :], s3[0:D1KB, 1, :]
                s1_pass(w2a, w2b, s2a, s2b)                  # + 2(g2 + e)
                s1_pass(w1a, w1b, s3a, s3b, stop=True)       # + g3 + e
                # z2 = wrap(v0 + bank); slot-1 rows 68:78 are exactly the
                # class zeros (0 + 0)
                return wwrap(ADD_WRAP_V, pab, xm, "z")

            def emit_st2(t01, z2, outst, fresh, fresh_ps):
                cs = slice((t01 % tpc) * BT, (t01 % tpc + 1) * BT)
                t0 = wsin(z2, "t0")
                t0a, t0b = t0[:, 0, :], t0[0:D2KB, 1, :]
                yab = psCD.tile([P, 2, BT], F32, tag="CD")
                if fresh_ps:
                    nc.scalar.memzero(yab[64:P, 1, :])
                ya, yb = yab[:, 0, :], yab[0:D2KB, 1, :]

                def s2_trim(wta, wtb, ra, rb, stop=False):
                    nc.tensor.matmul(yb, wta[:, P:D2E], ra,
                                     start=False, stop=stop,
                                     skip_group_check=True)
                    nc.tensor.matmul(yb, wtb[:, P:D2E], rb,
                                     start=False, stop=stop,
                                     skip_group_check=True)

                # g0' + e': ya's group opens and closes here (it is only
                # read afterwards); yb keeps accumulating
                nc.tensor.matmul(ya, v6a[:, 0:P], t0a,
                                 start=True, stop=False,
                                 skip_group_check=True)
                nc.tensor.matmul(ya, v6b[:, 0:P], t0b,
                                 start=False, stop=True,
                                 skip_group_check=True)
                nc.tensor.matmul(yb, v6a[:, P:D2E], t0a,
                                 start=True, stop=False,
                                 skip_group_check=True)
                nc.tensor.matmul(yb, v6b[:, P:D2E], t0b,
                                 start=False, stop=False,
                                 skip_group_check=True)
                n2 = wwrap(ADD_SCALE_WRAP_V, yab, z2[:], "2p", k=1.0 / 7.0)
                t2 = wsin(n2, "t2")
                t2a, t2b = t2[:, 0, :], t2[0:D2KB, 1, :]
                n3 = wwrap(ADD_SCALE_WRAP_V, yab, n2[:], "3p", k=2.0 / 7.0)
                t3 = wsin(n3, "t3")
                t3a, t3b = t3[:, 0, :], t3[0:D2KB, 1, :]
                s2_trim(v2a, v2b, t2a, t2b)                  # + 2g2'
                s2_trim(v1a, v1b, t3a, t3b, stop=True)       # + g3'
                nc.scalar.copy(outst[64:80, cs], yab[64:80, 1, :])
                nc.sync.dma_start(
                    out_d[:, (t01 // tpc) * ch + (t01 % tpc) * BT:
                          (t01 // tpc) * ch + (t01 % tpc + 1) * BT],
                    outst[64:80, cs])

            # software-pipelined emission: stage-1 of tile t runs ahead of
            # stage-2 of tile t-1 so each engine's in-order queue interleaves
            # the two stages instead of head-of-line blocking on tile t-1's
            # tail ops
            ntiles = nch * tpc
            chunks = {0: nxt}
            outsts = {}
            pend = {}
            for t in range(ntiles + 2):
                if t >= 2:
                    tp = t - 2
                    cp = tp // tpc
                    if tp % tpc == 0:
                        outsts[cp] = io.tile([80, ch], F32, tag="outst",
                                             name=f"outst{cp}")
                if t < ntiles:
                    c = t // tpc
                    if t % tpc == 0 and c + 1 < nch:
                        chunks[c + 1] = load_chunk(c + 1)
                    fresh1 = (not hoist) or t < 4
                    fps1 = (not hoist) or t < 2
                    pend[t] = emit_st1(t, chunks[c], fresh1, fps1)
                if t >= 2:
                    tp = t - 2
                    cp = tp // tpc
                    fresh2 = (not hoist) or tp < 4
                    fps2 = (not hoist) or tp < 2
                    emit_st2(tp, pend.pop(tp), outsts[cp], fresh2, fps2)
                    if tp % tpc == tpc - 1:
                        outsts.pop(cp)

    nc.compile()
    return nc


# ---- host side ------------------------------------------------------------

def _c2q(C):
    Q = 0.5 * (C + C.T)
    d = -Q.sum(axis=0)
    Q = Q.copy()
    Q[np.diag_indices_from(Q)] = d
    return Q


def _permute_v(v):
    """Stage-2 weight layout for the wide-packed state: contraction rows
    [state 0:196 | e | class 196:206] (the e-row rides the 0.5-driver at
    slot-1 row 68), target cols [0:196 | dummy zero | class] so the
    class targets land at slot-1 partitions 69:79."""
    rows = np.concatenate([v[0:D2 - NOUT], v[D2:D2 + 1],
                           v[D2 - NOUT:D2]], axis=0)
    return np.insert(rows, D2 - NOUT, 0.0, axis=1)


def _host_weights(fc_w, fc_b, qn, dim):
    """v-units dynamics matrix (scaled by 1.1*DT^2/pi), bias as last row."""
    W = (SC * DT2 / PI) * (_c2q(np.asarray(fc_w, np.float64))
                           + np.asarray(qn, np.float64) - np.eye(dim))
    eb = (SC * DT2 / PI) * np.asarray(fc_b, np.float64)
    return np.concatenate([W, eb[None, :]], axis=0)


def kernel(x, fc1_w, fc1_b, fc2_w, fc2_b, output_fac,
           Q_noise_small, Q_noise_large):
    global LAST_RESULTS
    if "nc" not in _CACHE:
        _CACHE["nc"] = _build_program()
    nc = _CACHE["nc"]

    w1 = _host_weights(fc1_w, fc1_b, Q_noise_small, D1)
    v1 = _host_weights(fc2_w, fc2_b, Q_noise_large, D2)
    v1 = _permute_v(v1)

    BF = ml_dtypes.bfloat16

    def bf(a):
        return np.ascontiguousarray(np.asarray(a, np.float32).astype(BF))

    # v0 = 1.1*x/pi wrapped into [-1, 1]; s0 = sin(1.1 x) exactly.  Both
    # ship wide-packed [128, 2, B]: slot 0 = state rows 0:128, slot 1 =
    # rows 128:196 (+ a ones row in s0 for the folded e-bias, zero pad)
    u = (SC / PI) * np.asarray(x, np.float64)
    u = u - 2.0 * ((u > 1.0).astype(np.float64) - (u < -1.0).astype(np.float64))
    vt = np.asarray(u.T, np.float32)                   # [D1, B]
    st = np.sin(PI * np.asarray(u.T, np.float64)).astype(np.float32)
    nb = vt.shape[1]
    vm = np.zeros((P, 2, nb), np.float32)
    vm[:, 0, :] = vt[0:P]
    vm[0:D1B, 1, :] = vt[P:D1]
    vm[D1B, 1, :] = 0.5          # ones-driver: every wrap keeps it 0.5,
    sm = np.zeros((P, 2, nb), np.float32)   # every sin turns it into 1.0
    sm[:, 0, :] = st[0:P]
    sm[0:D1B, 1, :] = st[P:D1]
    sm[D1B, 1, :] = 1.0

    common = {
        "w1": bf(w1), "w2": bf(2.0 * w1), "w6": bf(7.0 * w1),
        "v1": bf(v1), "v2": bf(2.0 * v1), "v6": bf(7.0 * v1),
    }
    in_maps = []
    for c in range(N_CORES):
        m = dict(common)
        m["v0"] = np.ascontiguousarray(vm[:, :, c * BC:(c + 1) * BC])
        m["s0"] = np.ascontiguousarray(sm[:, :, c * BC:(c + 1) * BC]).astype(BF)
        in_maps.append(m)

    res = None
    last_exc = None
    for _attempt in range(3):
        try:
            res = run_bass_kernel_spmd(
                nc, in_maps, core_ids=list(range(N_CORES)), trace=TRACE)
            break
        except Exception as e:  # transient NRT/device hiccups
            last_exc = e
            try:
                import time as _time

                import jax as _jax
                _jax.clear_caches()
                if hasattr(_jax, "clear_backends"):
                    _jax.clear_backends()
                _time.sleep(5)
            except Exception:
                pass
    if res is None:
        raise last_exc
    LAST_RESULTS = res

    out = np.empty((B, NOUT), np.float32)
    for c in range(N_CORES):
        out[c * BC:(c + 1) * BC, :] = res.results[c]["out"][5:15, :].T
    fac = float(np.asarray(output_fac)) * PI / SC
    return out * np.float32(fac)
